# revision 19
# baseline (speedup 1.0000x reference)
"""Trainium2 Bass kernel for nn_ActorCritic (GINE-style GNN actor-critic).

Strategy (sharding per spec hint): nodes/graphs sharded into 8 contiguous
slabs (core c owns nodes [c*32768, (c+1)*32768) == graphs [c*64, (c+1)*64));
edges sharded by the slab of their dst node so segment-sums stay local to a
slab; small weight matrices replicated.

Device (SPMD across 8 NeuronCores, one bass/Tile NEFF per stage shape):
  - MSG stage   : msgT = relu(gT + We.T @ eaT + be)      [128, E_pad] per core
  - NODE stage  : hT   = relu(W.T @ (h+agg).T + b)       [128, 32768] per core
  - HEADS stage : z=relu(laW1.T@embT+lab1); loc_logits=laW2.T@z+lab2;
                  pooled (per-graph sum); loc_values = critic MLP(pooled)
CPU does only index plumbing between stages (edge gathers h[src], per-slab
segment-sum by dst, Gumbel sampling glue) — all O(E*d) data movement, no
dense matmul work.
"""

import os
import sys

# The bass SPMD path targets the 8 axon-tunneled NeuronCores via jax's
# default platform; make sure axon wins over cpu when the host process
# hasn't pinned JAX_PLATFORMS itself.
os.environ.setdefault("JAX_PLATFORMS", "axon,cpu")

import numpy as np

sys.path.insert(0, "/opt/trn_rl_repo")

import concourse.bass as bass
import concourse.bacc as bacc
import concourse.mybir as mybir
from concourse import tile
from concourse.bass_utils import run_bass_kernel_spmd

N = 262144
E = 1048576
B = 512
IN, H, EDIM, AH, CH, NB = 8, 128, 4, 128, 128, 4
NCORES = 8
NC_N = N // NCORES          # 32768 nodes per core
NC_B = B // NCORES          # 64 graphs per core
SEG = N // B                # 512 nodes per graph
E_PAD = 155648              # per-core edge slab, even-run padded (38 * 4096)
CHUNK = 512

F32 = mybir.dt.float32

_kernel_cache = {}


def _mk_nc():
    return bacc.Bacc(
        "TRN2", target_bir_lowering=False, debug=False, num_devices=NCORES
    )


def _build_msg_kernel(e_pad, din):
    """msgT = relu(gT + We.T @ eaT + be_col)  -> [din<=128, e_pad]"""
    nc = _mk_nc()
    gT = nc.declare_dram_parameter("gT", [128, e_pad], F32, isOutput=False)
    eaT = nc.declare_dram_parameter("eaT", [EDIM, e_pad], F32, isOutput=False)
    We = nc.declare_dram_parameter("We", [EDIM, 128], F32, isOutput=False)
    be = nc.declare_dram_parameter("be", [128, 1], F32, isOutput=False)
    out = nc.declare_dram_parameter("msgT", [128, e_pad // 2], F32, isOutput=True)

    big = 4096
    nsub = big // CHUNK
    nchunk = e_pad // big
    with tile.TileContext(nc) as tc:
        with (
            tc.tile_pool(name="const", bufs=1) as cpool,
            tc.tile_pool(name="io", bufs=3) as io,
            tc.tile_pool(name="eap", bufs=2) as eap,
            tc.tile_pool(name="sm", bufs=6) as sm,
            tc.tile_pool(name="ps", bufs=4, space="PSUM") as ps,
        ):
            We_sb = cpool.tile([EDIM, 128], F32, tag="we")
            nc.sync.dma_start(We_sb[:], We[:, :])
            be_sb = cpool.tile([128, 1], F32, tag="be")
            nc.sync.dma_start(be_sb[:], be[:, :])
            for i in range(nchunk):
                ea_t = eap.tile([EDIM, big], F32, tag="ea")
                nc.sync.dma_start(ea_t[:], eaT[:, i * big:(i + 1) * big])
                g_t = io.tile([128, big], F32, tag="g")
                nc.sync.dma_start(g_t[:], gT[:, i * big:(i + 1) * big])
                o_t = io.tile([128, big], F32, tag="o")
                for j in range(nsub):
                    sl = slice(j * CHUNK, (j + 1) * CHUNK)
                    c_ps = ps.tile([128, CHUNK], F32, tag="c")
                    nc.tensor.matmul(
                        c_ps[:], We_sb[:], ea_t[:, sl], start=True, stop=True
                    )
                    s_t = sm.tile([128, CHUNK], F32, tag="s")
                    nc.vector.tensor_add(s_t[:], g_t[:, sl], c_ps[:])
                    nc.scalar.activation(
                        o_t[:, sl], s_t[:], mybir.ActivationFunctionType.Relu,
                        bias=be_sb[:, 0:1], scale=1.0,
                    )
                o3 = o_t[:].rearrange("p (k two) -> p k two", two=2)
                ph = io.tile([128, big // 2], F32, tag="oh")
                ph3 = ph[:].rearrange("p (k one) -> p k one", one=1)
                nc.vector.tensor_add(ph3, o3[:, :, 0:1], o3[:, :, 1:2])
                nc.scalar.dma_start(
                    out[:, i * (big // 2):(i + 1) * (big // 2)], ph[:]
                )
    nc.compile()
    return nc


def _build_node_kernel(din):
    """hT = relu(W.T @ sT + b_col) -> [128, NC_N];  sT = (h+agg).T  [din, NC_N]"""
    nc = _mk_nc()
    sT = nc.declare_dram_parameter("sT", [din, NC_N], F32, isOutput=False)
    W = nc.declare_dram_parameter("W", [din, 128], F32, isOutput=False)
    b = nc.declare_dram_parameter("b", [128, 1], F32, isOutput=False)
    out = nc.declare_dram_parameter("hT", [128, NC_N], F32, isOutput=True)

    big = 4096
    nsub = big // CHUNK
    nchunk = NC_N // big
    with tile.TileContext(nc) as tc:
        with (
            tc.tile_pool(name="const", bufs=1) as cpool,
            tc.tile_pool(name="io", bufs=3) as io,
            tc.tile_pool(name="ps", bufs=4, space="PSUM") as ps,
        ):
            W_sb = cpool.tile([din, 128], F32, tag="w")
            nc.sync.dma_start(W_sb[:], W[:, :])
            b_sb = cpool.tile([128, 1], F32, tag="b")
            nc.sync.dma_start(b_sb[:], b[:, :])
            whole = None
            if din <= 16:  # tiny input: load all of sT once
                whole = cpool.tile([din, NC_N], F32, tag="whole")
                nc.sync.dma_start(whole[:], sT[:, :])
            for i in range(nchunk):
                if whole is None:
                    s_t = io.tile([din, big], F32, tag="s")
                    nc.sync.dma_start(s_t[:], sT[:, i * big:(i + 1) * big])
                o_t = io.tile([128, big], F32, tag="o")
                for j in range(nsub):
                    sl = slice(j * CHUNK, (j + 1) * CHUNK)
                    gsl = slice(i * big + j * CHUNK, i * big + (j + 1) * CHUNK)
                    h_ps = ps.tile([128, CHUNK], F32, tag="h")
                    nc.tensor.matmul(
                        h_ps[:], W_sb[:],
                        whole[:, gsl] if whole is not None else s_t[:, sl],
                        start=True, stop=True,
                    )
                    nc.scalar.activation(
                        o_t[:, sl], h_ps[:], mybir.ActivationFunctionType.Relu,
                        bias=b_sb[:, 0:1], scale=1.0,
                    )
                nc.scalar.dma_start(out[:, i * big:(i + 1) * big], o_t[:])
    nc.compile()
    return nc


def _build_heads_kernel():
    """loc_logits = laW2.T@relu(laW1.T@embT+lab1)+lab2  [1, NC_N]
       pooled[g]  = sum_nodes embT per graph            [128, NC_B]
       loc_values = lcW2.T@relu(lcW1.T@pooled+lcb1)+lcb2 [1, NC_B]"""
    nc = _mk_nc()
    embT = nc.declare_dram_parameter("embT", [128, NC_N], F32, isOutput=False)
    laW1 = nc.declare_dram_parameter("laW1", [128, AH], F32, isOutput=False)
    lab1 = nc.declare_dram_parameter("lab1", [AH, 1], F32, isOutput=False)
    laW2 = nc.declare_dram_parameter("laW2", [AH, 1], F32, isOutput=False)
    lcW1 = nc.declare_dram_parameter("lcW1", [128, CH], F32, isOutput=False)
    lcb1 = nc.declare_dram_parameter("lcb1", [CH, 1], F32, isOutput=False)
    lcW2 = nc.declare_dram_parameter("lcW2", [CH, 1], F32, isOutput=False)
    scal = nc.declare_dram_parameter("scal", [1, 2], F32, isOutput=False)  # lab2, lcb2
    logits = nc.declare_dram_parameter("logits", [1, NC_N], F32, isOutput=True)
    values = nc.declare_dram_parameter("values", [1, NC_B], F32, isOutput=True)

    nchunk = NC_N // SEG  # one graph per chunk
    with tile.TileContext(nc) as tc:
        with (
            tc.tile_pool(name="const", bufs=1) as cpool,
            tc.tile_pool(name="io", bufs=4) as io,
            tc.tile_pool(name="acc", bufs=1) as acc,
            tc.tile_pool(name="ps", bufs=2, space="PSUM") as ps,
        ):
            laW1_sb = cpool.tile([128, AH], F32, tag="law1")
            nc.sync.dma_start(laW1_sb[:], laW1[:, :])
            lab1_sb = cpool.tile([AH, 1], F32, tag="lab1")
            nc.sync.dma_start(lab1_sb[:], lab1[:, :])
            laW2_sb = cpool.tile([AH, 1], F32, tag="law2")
            nc.sync.dma_start(laW2_sb[:], laW2[:, :])
            lcW1_sb = cpool.tile([128, CH], F32, tag="lcw1")
            nc.sync.dma_start(lcW1_sb[:], lcW1[:, :])
            lcb1_sb = cpool.tile([CH, 1], F32, tag="lcb1")
            nc.sync.dma_start(lcb1_sb[:], lcb1[:, :])
            lcW2_sb = cpool.tile([CH, 1], F32, tag="lcw2")
            nc.sync.dma_start(lcW2_sb[:], lcW2[:, :])
            scal_sb = cpool.tile([1, 2], F32, tag="scal")
            nc.sync.dma_start(scal_sb[:], scal[:, :])

            pooled = acc.tile([128, NC_B], F32, tag="pooled")
            for i in range(nchunk):
                e_t = io.tile([128, SEG], F32, tag="e")
                nc.sync.dma_start(e_t[:], embT[:, i * SEG:(i + 1) * SEG])
                z_ps = ps.tile([AH, SEG], F32, tag="z")
                nc.tensor.matmul(z_ps[:], laW1_sb[:], e_t[:], start=True, stop=True)
                z_t = io.tile([AH, SEG], F32, tag="zs")
                nc.scalar.activation(
                    z_t[:], z_ps[:], mybir.ActivationFunctionType.Relu,
                    bias=lab1_sb[:, 0:1], scale=1.0,
                )
                l_ps = ps.tile([1, SEG], F32, tag="l")
                nc.tensor.matmul(l_ps[:], laW2_sb[:], z_t[:], start=True, stop=True)
                l_t = io.tile([1, SEG], F32, tag="ls")
                nc.scalar.activation(
                    l_t[:], l_ps[:], mybir.ActivationFunctionType.Identity,
                    bias=scal_sb[0:1, 0:1], scale=1.0,
                )
                nc.sync.dma_start(logits[:, i * SEG:(i + 1) * SEG], l_t[:])
                nc.vector.reduce_sum(
                    pooled[:, i:i + 1], e_t[:], axis=mybir.AxisListType.X
                )
            v_ps = ps.tile([CH, NC_B], F32, tag="v1")
            nc.tensor.matmul(v_ps[:], lcW1_sb[:], pooled[:], start=True, stop=True)
            v_t = io.tile([CH, NC_B], F32, tag="v1s")
            nc.scalar.activation(
                v_t[:], v_ps[:], mybir.ActivationFunctionType.Relu,
                bias=lcb1_sb[:, 0:1], scale=1.0,
            )
            v2_ps = ps.tile([1, NC_B], F32, tag="v2")
            nc.tensor.matmul(v2_ps[:], lcW2_sb[:], v_t[:], start=True, stop=True)
            v2_t = io.tile([1, NC_B], F32, tag="v2s")
            nc.scalar.activation(
                v2_t[:], v2_ps[:], mybir.ActivationFunctionType.Identity,
                bias=scal_sb[0:1, 1:2], scale=1.0,
            )
            nc.sync.dma_start(values[:, :], v2_t[:])
    nc.compile()
    return nc


def _build_node_heads_kernel():
    """Fused final GNN layer + actor/critic heads.

    hT = relu(W.T @ sT + b); loc_logits = laW2.T@relu(laW1.T@hT+lab1)+lab2;
    pooled per graph; loc_values = lcW2.T@relu(lcW1.T@pooled+lcb1)+lcb2.
    """
    nc = _mk_nc()
    sT = nc.declare_dram_parameter("sT", [H, NC_N], F32, isOutput=False)
    W = nc.declare_dram_parameter("W", [H, 128], F32, isOutput=False)
    b = nc.declare_dram_parameter("b", [128, 1], F32, isOutput=False)
    laW1 = nc.declare_dram_parameter("laW1", [128, AH], F32, isOutput=False)
    lab1 = nc.declare_dram_parameter("lab1", [AH, 1], F32, isOutput=False)
    laW2 = nc.declare_dram_parameter("laW2", [AH, 1], F32, isOutput=False)
    lcW1 = nc.declare_dram_parameter("lcW1", [128, CH], F32, isOutput=False)
    lcb1 = nc.declare_dram_parameter("lcb1", [CH, 1], F32, isOutput=False)
    lcW2 = nc.declare_dram_parameter("lcW2", [CH, 1], F32, isOutput=False)
    scal = nc.declare_dram_parameter("scal", [1, 2], F32, isOutput=False)
    hT_o = nc.declare_dram_parameter("hT", [128, NC_N], F32, isOutput=True)
    logits = nc.declare_dram_parameter("logits", [1, NC_N], F32, isOutput=True)
    values = nc.declare_dram_parameter("values", [1, NC_B], F32, isOutput=True)

    nchunk = NC_N // SEG
    with tile.TileContext(nc) as tc:
        with (
            tc.tile_pool(name="const", bufs=1) as cpool,
            tc.tile_pool(name="io", bufs=4) as io,
            tc.tile_pool(name="acc", bufs=1) as acc,
            tc.tile_pool(name="psA", bufs=2, space="PSUM") as psA,
            tc.tile_pool(name="psB", bufs=2, space="PSUM") as psB,
            tc.tile_pool(name="psC", bufs=1, space="PSUM") as psC,
        ):
            def ld(name, shape):
                t = cpool.tile(shape, F32, tag=name)
                nc.sync.dma_start(t[:], {
                    "w": W, "b": b, "law1": laW1, "lab1": lab1, "law2": laW2,
                    "lcw1": lcW1, "lcb1": lcb1, "lcw2": lcW2, "scal": scal,
                }[name][:, :])
                return t

            W_sb = ld("w", [H, 128])
            b_sb = ld("b", [128, 1])
            laW1_sb = ld("law1", [128, AH])
            lab1_sb = ld("lab1", [AH, 1])
            laW2_sb = ld("law2", [AH, 1])
            lcW1_sb = ld("lcw1", [128, CH])
            lcb1_sb = ld("lcb1", [CH, 1])
            lcW2_sb = ld("lcw2", [CH, 1])
            scal_sb = ld("scal", [1, 2])

            pooled = acc.tile([128, NC_B], F32, tag="pooled")
            big = 4096
            nsub = big // SEG
            for i in range(NC_N // big):
                s_t = io.tile([H, big], F32, tag="s")
                nc.sync.dma_start(s_t[:], sT[:, i * big:(i + 1) * big])
                h_t = io.tile([128, big], F32, tag="h_sb")
                l_t = io.tile([1, big], F32, tag="l_sb")
                for j in range(nsub):
                    sl = slice(j * SEG, (j + 1) * SEG)
                    h_ps = psA.tile([128, SEG], F32, tag="h")
                    nc.tensor.matmul(
                        h_ps[:], W_sb[:], s_t[:, sl], start=True, stop=True
                    )
                    nc.scalar.activation(
                        h_t[:, sl], h_ps[:], mybir.ActivationFunctionType.Relu,
                        bias=b_sb[:, 0:1], scale=1.0,
                    )
                    z_ps = psA.tile([AH, SEG], F32, tag="z")
                    nc.tensor.matmul(
                        z_ps[:], laW1_sb[:], h_t[:, sl], start=True, stop=True
                    )
                    z_t = io.tile([AH, SEG], F32, tag="z_sb")
                    nc.scalar.activation(
                        z_t[:], z_ps[:], mybir.ActivationFunctionType.Relu,
                        bias=lab1_sb[:, 0:1], scale=1.0,
                    )
                    l_ps = psB.tile([1, SEG], F32, tag="l")
                    nc.tensor.matmul(
                        l_ps[:], laW2_sb[:], z_t[:], start=True, stop=True
                    )
                    nc.scalar.activation(
                        l_t[:, sl], l_ps[:], mybir.ActivationFunctionType.Identity,
                        bias=scal_sb[0:1, 0:1], scale=1.0,
                    )
                    nc.vector.reduce_sum(
                        pooled[:, i * nsub + j:i * nsub + j + 1], h_t[:, sl],
                        axis=mybir.AxisListType.X,
                    )
                nc.scalar.dma_start(hT_o[:, i * big:(i + 1) * big], h_t[:])
                nc.scalar.dma_start(logits[:, i * big:(i + 1) * big], l_t[:])
            v_ps = psC.tile([CH, NC_B], F32, tag="v1")
            nc.tensor.matmul(v_ps[:], lcW1_sb[:], pooled[:], start=True, stop=True)
            v_t = io.tile([CH, NC_B], F32, tag="v1s")
            nc.scalar.activation(
                v_t[:], v_ps[:], mybir.ActivationFunctionType.Relu,
                bias=lcb1_sb[:, 0:1], scale=1.0,
            )
            v2_ps = psC.tile([1, NC_B], F32, tag="v2")
            nc.tensor.matmul(v2_ps[:], lcW2_sb[:], v_t[:], start=True, stop=True)
            v2_t = io.tile([1, NC_B], F32, tag="v2s")
            nc.scalar.activation(
                v2_t[:], v2_ps[:], mybir.ActivationFunctionType.Identity,
                bias=scal_sb[0:1, 1:2], scale=1.0,
            )
            nc.sync.dma_start(values[:, :], v2_t[:])
    nc.compile()
    return nc


def _get(name, builder, *args):
    if name not in _kernel_cache:
        _kernel_cache[name] = builder(*args)
    return _kernel_cache[name]


_EXEC_NS = []
_TRACE = os.environ.get("BASS_KERNEL_TRACE", "0") == "1"


def _run(nc, in_maps):
    if _TRACE:
        try:
            res = run_bass_kernel_spmd(
                nc, in_maps, list(range(NCORES)), trace=True
            )
            if res.exec_time_ns:
                _EXEC_NS.append(res.exec_time_ns)
            return res.results
        except Exception as e:  # trace infra unavailable — fall back untraced
            print(f"trace failed ({type(e).__name__}: {e}); untraced rerun",
                  file=sys.stderr)
    res = run_bass_kernel_spmd(nc, in_maps, list(range(NCORES)))
    if res.exec_time_ns:
        _EXEC_NS.append(res.exec_time_ns)
    return res.results


def _seg_sum_by_dst(msg, dst_local, ncol):
    out = np.zeros((NC_N, ncol), dtype=np.float32)
    np.add.at(out, dst_local, msg)
    return out


def kernel(**inputs):
    x = np.asarray(inputs["x"], np.float32)
    edge_index = np.asarray(inputs["edge_index"])
    edge_attr = np.asarray(inputs["edge_attr"], np.float32)

    Wes = [np.asarray(inputs[f"We{l}"], np.float32) for l in range(3)]
    bes = [np.asarray(inputs[f"be{l}"], np.float32) for l in range(3)]
    Ws = [np.asarray(inputs[f"W{l}"], np.float32) for l in range(3)]
    bs = [np.asarray(inputs[f"b{l}"], np.float32) for l in range(3)]

    src = np.asarray(edge_index[0], np.int64)
    dst = np.asarray(edge_index[1], np.int64)

    # ---- shard edges by dst slab, dst-sorted, runs padded to even ----------
    # Each dst's edge run is padded to even length (pad edges gather a -1e9
    # sentinel column -> msg 0), so the device can pair-reduce adjacent msg
    # columns and halve the DMA-bound msgT output stream.
    order = np.argsort(dst, kind="stable")
    core_of = dst // NC_N
    counts = np.bincount(core_of, minlength=NCORES)
    starts = np.zeros(NCORES + 1, np.int64)
    starts[1:] = np.cumsum(counts)
    src_s = src[order]
    dst_s = dst[order]
    ea_s = edge_attr[order]

    e_src = np.full((NCORES, E_PAD), N, np.int64)  # N = sentinel column
    pair_dst = np.zeros((NCORES, E_PAD // 2), np.int64)
    eaT_sh = np.zeros((NCORES, EDIM, E_PAD), np.float32)
    for c in range(NCORES):
        n = counts[c]
        sl = slice(starts[c], starts[c + 1])
        d = (dst_s[sl] - c * NC_N).astype(np.int64)
        deg = np.bincount(d, minlength=NC_N)
        pdeg = deg + (deg & 1)
        run_start = np.concatenate(([0], np.cumsum(deg)))
        prun_start = np.concatenate(([0], np.cumsum(pdeg)))
        assert prun_start[-1] <= E_PAD, prun_start[-1]
        pos = np.arange(n) - run_start[d] + prun_start[d]
        e_src[c, pos] = src_s[sl]
        eaT_sh[c, :, pos] = ea_s[sl]
        pair_dst[c, pos // 2] = d

    # ---- layer 0 (d=8): msg on CPU (tiny), node update on device -----------
    c0 = edge_attr @ Wes[0] + bes[0]
    msg0 = np.maximum(x[src] + c0, 0.0)
    agg0 = np.zeros((N, IN), np.float32)
    np.add.at(agg0, dst, msg0)
    s0 = x + agg0

    nodek8 = _get("node8", _build_node_kernel, IN)
    in_maps = [
        {
            "sT": np.ascontiguousarray(s0[c * NC_N:(c + 1) * NC_N].T),
            "W": Ws[0],
            "b": bs[0].reshape(128, 1),
        }
        for c in range(NCORES)
    ]
    res = _run(nodek8, in_maps)
    hT = np.concatenate([res[c]["hT"] for c in range(NCORES)], axis=1)  # [128, N]

    # ---- layers 1,2: msg on device, segsum on CPU, node on device ----------
    msgk = _get("msg", _build_msg_kernel, E_PAD, H)
    nodek = _get("node128", _build_node_kernel, H)
    nodeheadsk = _get("nodeheads", _build_node_heads_kernel)
    head_consts = {
        "laW1": np.asarray(inputs["laW1"], np.float32),
        "lab1": np.asarray(inputs["lab1"], np.float32).reshape(AH, 1),
        "laW2": np.asarray(inputs["laW2"], np.float32).reshape(AH, 1),
        "lcW1": np.asarray(inputs["lcW1"], np.float32),
        "lcb1": np.asarray(inputs["lcb1"], np.float32).reshape(CH, 1),
        "lcW2": np.asarray(inputs["lcW2"], np.float32).reshape(CH, 1),
        "scal": np.array(
            [[np.float32(inputs["lab2"][0]), np.float32(inputs["lcb2"][0])]],
            np.float32,
        ),
    }
    res = None
    for l in (1, 2):
        in_maps = []
        hT_ext = np.concatenate(
            [hT, np.full((128, 1), -1e9, np.float32)], axis=1
        )
        for c in range(NCORES):
            gT = np.ascontiguousarray(hT_ext[:, e_src[c]])  # [128, E_PAD]
            in_maps.append({
                "gT": gT,
                "eaT": eaT_sh[c],
                "We": Wes[l],
                "be": bes[l].reshape(128, 1),
            })
        res = _run(msgk, in_maps)
        in_maps = []
        for c in range(NCORES):
            msgT = res[c]["msgT"]  # [128, E_PAD // 2] pair-reduced
            agg = _seg_sum_by_dst(msgT.T, pair_dst[c], H)
            s_slab = hT[:, c * NC_N:(c + 1) * NC_N].T + agg
            m = {
                "sT": np.ascontiguousarray(s_slab.T),
                "W": Ws[l],
                "b": bs[l].reshape(128, 1),
            }
            if l == 2:
                m.update(head_consts)
            in_maps.append(m)
        res = _run(nodek if l == 1 else nodeheadsk, in_maps)
        hT = np.concatenate([res[c]["hT"] for c in range(NCORES)], axis=1)

    emb = hT.T  # [N, 128]
    loc_logits = np.concatenate(
        [res[c]["logits"][0] for c in range(NCORES)]
    ).astype(np.float32)
    loc_values = np.concatenate(
        [res[c]["values"][0] for c in range(NCORES)]
    ).astype(np.float32)

    # ---- segmented softmax / sampling glue (CPU, mirrors reference ops) ----
    import jax
    import jax.numpy as jnp

    _cpu = jax.devices("cpu")[0]

    lg = loc_logits.reshape(B, SEG)
    seg_max = lg.max(axis=1)
    shifted = (lg - seg_max[:, None]).astype(np.float32)
    ex = np.exp(shifted, dtype=np.float32)
    denom = ex.sum(axis=1, dtype=np.float32)
    logp = (shifted - np.log(denom, dtype=np.float32)[:, None]).astype(np.float32)
    p = np.exp(logp, dtype=np.float32)
    loc_entropy = (-(p * logp).sum(axis=1, dtype=np.float32)).astype(np.float32)

    with jax.default_device(_cpu):
        key = jax.random.key(42)
        u = np.asarray(
            jax.random.uniform(key, (N,), minval=1e-6, maxval=1.0 - 1e-6), np.float32
        )
    gnoise = -np.log(-np.log(u, dtype=np.float32), dtype=np.float32)
    pert = (loc_logits + gnoise).astype(np.float32).reshape(B, SEG)
    pmax = pert.max(axis=1)
    arange = np.arange(N, dtype=np.int64).reshape(B, SEG)
    cand = np.where(pert >= pmax[:, None], arange, N)
    locations = cand.min(axis=1).astype(np.int32)
    loc_log_probs = logp.reshape(-1)[locations].astype(np.float32)

    sel = emb[locations]  # [B, 128]
    maW1 = np.asarray(inputs["maW1"], np.float32)
    mab1 = np.asarray(inputs["mab1"], np.float32)
    maW2 = np.asarray(inputs["maW2"], np.float32)
    mab2 = np.asarray(inputs["mab2"], np.float32)
    mut_logits = np.maximum(sel @ maW1 + mab1, 0.0) @ maW2 + mab2
    mlg = mut_logits - mut_logits.max(axis=1, keepdims=True)
    mex = np.exp(mlg, dtype=np.float32)
    mut_logp = (mlg - np.log(mex.sum(axis=1, keepdims=True, dtype=np.float32))).astype(
        np.float32
    )
    with jax.default_device(_cpu):
        mutations = np.asarray(
            jax.random.categorical(
                jax.random.fold_in(key, 1), jnp.asarray(mut_logits), axis=-1
            ),
            np.int32,
        )
    mut_log_probs = np.take_along_axis(
        mut_logp, mutations[:, None].astype(np.int64), axis=1
    )[:, 0].astype(np.float32)
    mp = np.exp(mut_logp, dtype=np.float32)
    mut_entropy = (-(mp * mut_logp).sum(axis=1, dtype=np.float32)).astype(np.float32)

    mcW1 = np.asarray(inputs["mcW1"], np.float32)
    mcb1 = np.asarray(inputs["mcb1"], np.float32)
    mcW2 = np.asarray(inputs["mcW2"], np.float32)
    mcb2 = np.asarray(inputs["mcb2"], np.float32)
    mut_values = (np.maximum(sel @ mcW1 + mcb1, 0.0) @ mcW2 + mcb2)[:, 0].astype(
        np.float32
    )

    return (
        locations,
        np.asarray(mutations, np.int32),
        loc_log_probs,
        mut_log_probs,
        loc_entropy,
        mut_entropy,
        loc_values,
        mut_values,
        emb.astype(np.float32),
    )


# revision 21
# speedup vs baseline: 1.0823x; 1.0823x over previous
"""Trainium2 Bass kernel for nn_ActorCritic (GINE-style GNN actor-critic).

Strategy (sharding per spec hint): nodes/graphs sharded into 8 contiguous
slabs (core c owns nodes [c*32768, (c+1)*32768) == graphs [c*64, (c+1)*64));
edges sharded by the slab of their dst node so segment-sums stay local to a
slab; small weight matrices replicated.

Device (SPMD across 8 NeuronCores, one bass/Tile NEFF per stage shape):
  - MSG stage   : msgT = relu(gT + We.T @ eaT + be)      [128, E_pad] per core
  - NODE stage  : hT   = relu(W.T @ (h+agg).T + b)       [128, 32768] per core
  - HEADS stage : z=relu(laW1.T@embT+lab1); loc_logits=laW2.T@z+lab2;
                  pooled (per-graph sum); loc_values = critic MLP(pooled)
CPU does only index plumbing between stages (edge gathers h[src], per-slab
segment-sum by dst, Gumbel sampling glue) — all O(E*d) data movement, no
dense matmul work.
"""

import os
import sys

# The bass SPMD path targets the 8 axon-tunneled NeuronCores via jax's
# default platform; make sure axon wins over cpu when the host process
# hasn't pinned JAX_PLATFORMS itself.
os.environ.setdefault("JAX_PLATFORMS", "axon,cpu")

import numpy as np

sys.path.insert(0, "/opt/trn_rl_repo")

import concourse.bass as bass
import concourse.bacc as bacc
import concourse.mybir as mybir
from concourse import tile
from concourse.bass_utils import run_bass_kernel_spmd

N = 262144
E = 1048576
B = 512
IN, H, EDIM, AH, CH, NB = 8, 128, 4, 128, 128, 4
NCORES = 8
NC_N = N // NCORES          # 32768 nodes per core
NC_B = B // NCORES          # 64 graphs per core
SEG = N // B                # 512 nodes per graph
E_PAD = 135168              # per-core edge slab, padded (264 * 512)
CHUNK = 512

F32 = mybir.dt.float32

_kernel_cache = {}


def _mk_nc():
    return bacc.Bacc(
        "TRN2", target_bir_lowering=False, debug=False, num_devices=NCORES
    )


def _build_msg_kernel(e_pad, din):
    """msgT = relu(gT + We.T @ eaT + be_col)  -> [din<=128, e_pad]"""
    nc = _mk_nc()
    gT = nc.declare_dram_parameter("gT", [128, e_pad], F32, isOutput=False)
    eaT = nc.declare_dram_parameter("eaT", [EDIM, e_pad], F32, isOutput=False)
    We = nc.declare_dram_parameter("We", [EDIM, 128], F32, isOutput=False)
    be = nc.declare_dram_parameter("be", [128, 1], F32, isOutput=False)
    out = nc.declare_dram_parameter("msgT", [128, e_pad], F32, isOutput=True)

    big = 4096
    nsub = big // CHUNK
    nchunk = e_pad // big
    with tile.TileContext(nc) as tc:
        with (
            tc.tile_pool(name="const", bufs=1) as cpool,
            tc.tile_pool(name="gp", bufs=4) as gp,
            tc.tile_pool(name="op", bufs=3) as op,
            tc.tile_pool(name="eap", bufs=2) as eap,
            tc.tile_pool(name="sm", bufs=6) as sm,
            tc.tile_pool(name="ps", bufs=8, space="PSUM") as ps,
        ):
            We_sb = cpool.tile([EDIM, 128], F32, tag="we")
            nc.sync.dma_start(We_sb[:], We[:, :])
            be_sb = cpool.tile([128, 1], F32, tag="be")
            nc.sync.dma_start(be_sb[:], be[:, :])
            for i in range(nchunk):
                ea_t = eap.tile([EDIM, big], F32, tag="ea")
                nc.sync.dma_start(ea_t[:], eaT[:, i * big:(i + 1) * big])
                g_t = gp.tile([128, big], F32, tag="g")
                nc.gpsimd.dma_start(g_t[:], gT[:, i * big:(i + 1) * big])
                o_t = op.tile([128, big], F32, tag="o")
                for j in range(nsub):
                    sl = slice(j * CHUNK, (j + 1) * CHUNK)
                    c_ps = ps.tile([128, CHUNK], F32, tag="c")
                    nc.tensor.matmul(
                        c_ps[:], We_sb[:], ea_t[:, sl], start=True, stop=True
                    )
                    s_t = sm.tile([128, CHUNK], F32, tag="s")
                    nc.vector.tensor_add(s_t[:], g_t[:, sl], c_ps[:])
                    nc.scalar.activation(
                        o_t[:, sl], s_t[:], mybir.ActivationFunctionType.Relu,
                        bias=be_sb[:, 0:1], scale=1.0,
                    )
                nc.scalar.dma_start(out[:, i * big:(i + 1) * big], o_t[:])
    nc.compile()
    return nc


def _build_node_kernel(din):
    """hT = relu(W.T @ sT + b_col) -> [128, NC_N];  sT = (h+agg).T  [din, NC_N]"""
    nc = _mk_nc()
    sT = nc.declare_dram_parameter("sT", [din, NC_N], F32, isOutput=False)
    W = nc.declare_dram_parameter("W", [din, 128], F32, isOutput=False)
    b = nc.declare_dram_parameter("b", [128, 1], F32, isOutput=False)
    out = nc.declare_dram_parameter("hT", [128, NC_N], F32, isOutput=True)

    big = 4096
    nsub = big // CHUNK
    nchunk = NC_N // big
    with tile.TileContext(nc) as tc:
        with (
            tc.tile_pool(name="const", bufs=1) as cpool,
            tc.tile_pool(name="io", bufs=3) as io,
            tc.tile_pool(name="ps", bufs=4, space="PSUM") as ps,
        ):
            W_sb = cpool.tile([din, 128], F32, tag="w")
            nc.sync.dma_start(W_sb[:], W[:, :])
            b_sb = cpool.tile([128, 1], F32, tag="b")
            nc.sync.dma_start(b_sb[:], b[:, :])
            whole = None
            if din <= 16:  # tiny input: load all of sT once
                whole = cpool.tile([din, NC_N], F32, tag="whole")
                nc.sync.dma_start(whole[:], sT[:, :])
            for i in range(nchunk):
                if whole is None:
                    s_t = io.tile([din, big], F32, tag="s")
                    nc.sync.dma_start(s_t[:], sT[:, i * big:(i + 1) * big])
                o_t = io.tile([128, big], F32, tag="o")
                for j in range(nsub):
                    sl = slice(j * CHUNK, (j + 1) * CHUNK)
                    gsl = slice(i * big + j * CHUNK, i * big + (j + 1) * CHUNK)
                    h_ps = ps.tile([128, CHUNK], F32, tag="h")
                    nc.tensor.matmul(
                        h_ps[:], W_sb[:],
                        whole[:, gsl] if whole is not None else s_t[:, sl],
                        start=True, stop=True,
                    )
                    nc.scalar.activation(
                        o_t[:, sl], h_ps[:], mybir.ActivationFunctionType.Relu,
                        bias=b_sb[:, 0:1], scale=1.0,
                    )
                nc.scalar.dma_start(out[:, i * big:(i + 1) * big], o_t[:])
    nc.compile()
    return nc


def _build_heads_kernel():
    """loc_logits = laW2.T@relu(laW1.T@embT+lab1)+lab2  [1, NC_N]
       pooled[g]  = sum_nodes embT per graph            [128, NC_B]
       loc_values = lcW2.T@relu(lcW1.T@pooled+lcb1)+lcb2 [1, NC_B]"""
    nc = _mk_nc()
    embT = nc.declare_dram_parameter("embT", [128, NC_N], F32, isOutput=False)
    laW1 = nc.declare_dram_parameter("laW1", [128, AH], F32, isOutput=False)
    lab1 = nc.declare_dram_parameter("lab1", [AH, 1], F32, isOutput=False)
    laW2 = nc.declare_dram_parameter("laW2", [AH, 1], F32, isOutput=False)
    lcW1 = nc.declare_dram_parameter("lcW1", [128, CH], F32, isOutput=False)
    lcb1 = nc.declare_dram_parameter("lcb1", [CH, 1], F32, isOutput=False)
    lcW2 = nc.declare_dram_parameter("lcW2", [CH, 1], F32, isOutput=False)
    scal = nc.declare_dram_parameter("scal", [1, 2], F32, isOutput=False)  # lab2, lcb2
    logits = nc.declare_dram_parameter("logits", [1, NC_N], F32, isOutput=True)
    values = nc.declare_dram_parameter("values", [1, NC_B], F32, isOutput=True)

    nchunk = NC_N // SEG  # one graph per chunk
    with tile.TileContext(nc) as tc:
        with (
            tc.tile_pool(name="const", bufs=1) as cpool,
            tc.tile_pool(name="io", bufs=4) as io,
            tc.tile_pool(name="acc", bufs=1) as acc,
            tc.tile_pool(name="ps", bufs=2, space="PSUM") as ps,
        ):
            laW1_sb = cpool.tile([128, AH], F32, tag="law1")
            nc.sync.dma_start(laW1_sb[:], laW1[:, :])
            lab1_sb = cpool.tile([AH, 1], F32, tag="lab1")
            nc.sync.dma_start(lab1_sb[:], lab1[:, :])
            laW2_sb = cpool.tile([AH, 1], F32, tag="law2")
            nc.sync.dma_start(laW2_sb[:], laW2[:, :])
            lcW1_sb = cpool.tile([128, CH], F32, tag="lcw1")
            nc.sync.dma_start(lcW1_sb[:], lcW1[:, :])
            lcb1_sb = cpool.tile([CH, 1], F32, tag="lcb1")
            nc.sync.dma_start(lcb1_sb[:], lcb1[:, :])
            lcW2_sb = cpool.tile([CH, 1], F32, tag="lcw2")
            nc.sync.dma_start(lcW2_sb[:], lcW2[:, :])
            scal_sb = cpool.tile([1, 2], F32, tag="scal")
            nc.sync.dma_start(scal_sb[:], scal[:, :])

            pooled = acc.tile([128, NC_B], F32, tag="pooled")
            for i in range(nchunk):
                e_t = io.tile([128, SEG], F32, tag="e")
                nc.sync.dma_start(e_t[:], embT[:, i * SEG:(i + 1) * SEG])
                z_ps = ps.tile([AH, SEG], F32, tag="z")
                nc.tensor.matmul(z_ps[:], laW1_sb[:], e_t[:], start=True, stop=True)
                z_t = io.tile([AH, SEG], F32, tag="zs")
                nc.scalar.activation(
                    z_t[:], z_ps[:], mybir.ActivationFunctionType.Relu,
                    bias=lab1_sb[:, 0:1], scale=1.0,
                )
                l_ps = ps.tile([1, SEG], F32, tag="l")
                nc.tensor.matmul(l_ps[:], laW2_sb[:], z_t[:], start=True, stop=True)
                l_t = io.tile([1, SEG], F32, tag="ls")
                nc.scalar.activation(
                    l_t[:], l_ps[:], mybir.ActivationFunctionType.Identity,
                    bias=scal_sb[0:1, 0:1], scale=1.0,
                )
                nc.sync.dma_start(logits[:, i * SEG:(i + 1) * SEG], l_t[:])
                nc.vector.reduce_sum(
                    pooled[:, i:i + 1], e_t[:], axis=mybir.AxisListType.X
                )
            v_ps = ps.tile([CH, NC_B], F32, tag="v1")
            nc.tensor.matmul(v_ps[:], lcW1_sb[:], pooled[:], start=True, stop=True)
            v_t = io.tile([CH, NC_B], F32, tag="v1s")
            nc.scalar.activation(
                v_t[:], v_ps[:], mybir.ActivationFunctionType.Relu,
                bias=lcb1_sb[:, 0:1], scale=1.0,
            )
            v2_ps = ps.tile([1, NC_B], F32, tag="v2")
            nc.tensor.matmul(v2_ps[:], lcW2_sb[:], v_t[:], start=True, stop=True)
            v2_t = io.tile([1, NC_B], F32, tag="v2s")
            nc.scalar.activation(
                v2_t[:], v2_ps[:], mybir.ActivationFunctionType.Identity,
                bias=scal_sb[0:1, 1:2], scale=1.0,
            )
            nc.sync.dma_start(values[:, :], v2_t[:])
    nc.compile()
    return nc


def _build_node_heads_kernel():
    """Fused final GNN layer + actor/critic heads.

    hT = relu(W.T @ sT + b); loc_logits = laW2.T@relu(laW1.T@hT+lab1)+lab2;
    pooled per graph; loc_values = lcW2.T@relu(lcW1.T@pooled+lcb1)+lcb2.
    """
    nc = _mk_nc()
    sT = nc.declare_dram_parameter("sT", [H, NC_N], F32, isOutput=False)
    W = nc.declare_dram_parameter("W", [H, 128], F32, isOutput=False)
    b = nc.declare_dram_parameter("b", [128, 1], F32, isOutput=False)
    laW1 = nc.declare_dram_parameter("laW1", [128, AH], F32, isOutput=False)
    lab1 = nc.declare_dram_parameter("lab1", [AH, 1], F32, isOutput=False)
    laW2 = nc.declare_dram_parameter("laW2", [AH, 1], F32, isOutput=False)
    lcW1 = nc.declare_dram_parameter("lcW1", [128, CH], F32, isOutput=False)
    lcb1 = nc.declare_dram_parameter("lcb1", [CH, 1], F32, isOutput=False)
    lcW2 = nc.declare_dram_parameter("lcW2", [CH, 1], F32, isOutput=False)
    scal = nc.declare_dram_parameter("scal", [1, 2], F32, isOutput=False)
    hT_o = nc.declare_dram_parameter("hT", [128, NC_N], F32, isOutput=True)
    logits = nc.declare_dram_parameter("logits", [1, NC_N], F32, isOutput=True)
    values = nc.declare_dram_parameter("values", [1, NC_B], F32, isOutput=True)

    nchunk = NC_N // SEG
    with tile.TileContext(nc) as tc:
        with (
            tc.tile_pool(name="const", bufs=1) as cpool,
            tc.tile_pool(name="io", bufs=4) as io,
            tc.tile_pool(name="acc", bufs=1) as acc,
            tc.tile_pool(name="psA", bufs=2, space="PSUM") as psA,
            tc.tile_pool(name="psB", bufs=2, space="PSUM") as psB,
            tc.tile_pool(name="psC", bufs=1, space="PSUM") as psC,
        ):
            def ld(name, shape):
                t = cpool.tile(shape, F32, tag=name)
                nc.sync.dma_start(t[:], {
                    "w": W, "b": b, "law1": laW1, "lab1": lab1, "law2": laW2,
                    "lcw1": lcW1, "lcb1": lcb1, "lcw2": lcW2, "scal": scal,
                }[name][:, :])
                return t

            W_sb = ld("w", [H, 128])
            b_sb = ld("b", [128, 1])
            laW1_sb = ld("law1", [128, AH])
            lab1_sb = ld("lab1", [AH, 1])
            laW2_sb = ld("law2", [AH, 1])
            lcW1_sb = ld("lcw1", [128, CH])
            lcb1_sb = ld("lcb1", [CH, 1])
            lcW2_sb = ld("lcw2", [CH, 1])
            scal_sb = ld("scal", [1, 2])

            pooled = acc.tile([128, NC_B], F32, tag="pooled")
            big = 4096
            nsub = big // SEG
            for i in range(NC_N // big):
                s_t = io.tile([H, big], F32, tag="s")
                nc.sync.dma_start(s_t[:], sT[:, i * big:(i + 1) * big])
                h_t = io.tile([128, big], F32, tag="h_sb")
                l_t = io.tile([1, big], F32, tag="l_sb")
                for j in range(nsub):
                    sl = slice(j * SEG, (j + 1) * SEG)
                    h_ps = psA.tile([128, SEG], F32, tag="h")
                    nc.tensor.matmul(
                        h_ps[:], W_sb[:], s_t[:, sl], start=True, stop=True
                    )
                    nc.scalar.activation(
                        h_t[:, sl], h_ps[:], mybir.ActivationFunctionType.Relu,
                        bias=b_sb[:, 0:1], scale=1.0,
                    )
                    z_ps = psA.tile([AH, SEG], F32, tag="z")
                    nc.tensor.matmul(
                        z_ps[:], laW1_sb[:], h_t[:, sl], start=True, stop=True
                    )
                    z_t = io.tile([AH, SEG], F32, tag="z_sb")
                    nc.scalar.activation(
                        z_t[:], z_ps[:], mybir.ActivationFunctionType.Relu,
                        bias=lab1_sb[:, 0:1], scale=1.0,
                    )
                    l_ps = psB.tile([1, SEG], F32, tag="l")
                    nc.tensor.matmul(
                        l_ps[:], laW2_sb[:], z_t[:], start=True, stop=True
                    )
                    nc.scalar.activation(
                        l_t[:, sl], l_ps[:], mybir.ActivationFunctionType.Identity,
                        bias=scal_sb[0:1, 0:1], scale=1.0,
                    )
                    nc.vector.reduce_sum(
                        pooled[:, i * nsub + j:i * nsub + j + 1], h_t[:, sl],
                        axis=mybir.AxisListType.X,
                    )
                nc.scalar.dma_start(hT_o[:, i * big:(i + 1) * big], h_t[:])
                nc.scalar.dma_start(logits[:, i * big:(i + 1) * big], l_t[:])
            v_ps = psC.tile([CH, NC_B], F32, tag="v1")
            nc.tensor.matmul(v_ps[:], lcW1_sb[:], pooled[:], start=True, stop=True)
            v_t = io.tile([CH, NC_B], F32, tag="v1s")
            nc.scalar.activation(
                v_t[:], v_ps[:], mybir.ActivationFunctionType.Relu,
                bias=lcb1_sb[:, 0:1], scale=1.0,
            )
            v2_ps = psC.tile([1, NC_B], F32, tag="v2")
            nc.tensor.matmul(v2_ps[:], lcW2_sb[:], v_t[:], start=True, stop=True)
            v2_t = io.tile([1, NC_B], F32, tag="v2s")
            nc.scalar.activation(
                v2_t[:], v2_ps[:], mybir.ActivationFunctionType.Identity,
                bias=scal_sb[0:1, 1:2], scale=1.0,
            )
            nc.sync.dma_start(values[:, :], v2_t[:])
    nc.compile()
    return nc


def _get(name, builder, *args):
    if name not in _kernel_cache:
        _kernel_cache[name] = builder(*args)
    return _kernel_cache[name]


_EXEC_NS = []
_TRACE = os.environ.get("BASS_KERNEL_TRACE", "0") == "1"


def _run(nc, in_maps):
    if _TRACE:
        try:
            res = run_bass_kernel_spmd(
                nc, in_maps, list(range(NCORES)), trace=True
            )
            if res.exec_time_ns:
                _EXEC_NS.append(res.exec_time_ns)
            return res.results
        except Exception as e:  # trace infra unavailable — fall back untraced
            print(f"trace failed ({type(e).__name__}: {e}); untraced rerun",
                  file=sys.stderr)
    res = run_bass_kernel_spmd(nc, in_maps, list(range(NCORES)))
    if res.exec_time_ns:
        _EXEC_NS.append(res.exec_time_ns)
    return res.results


def _seg_sum_by_dst(msg, dst_local, ncol):
    out = np.zeros((NC_N, ncol), dtype=np.float32)
    np.add.at(out, dst_local, msg)
    return out


def kernel(**inputs):
    x = np.asarray(inputs["x"], np.float32)
    edge_index = np.asarray(inputs["edge_index"])
    edge_attr = np.asarray(inputs["edge_attr"], np.float32)

    Wes = [np.asarray(inputs[f"We{l}"], np.float32) for l in range(3)]
    bes = [np.asarray(inputs[f"be{l}"], np.float32) for l in range(3)]
    Ws = [np.asarray(inputs[f"W{l}"], np.float32) for l in range(3)]
    bs = [np.asarray(inputs[f"b{l}"], np.float32) for l in range(3)]

    src = np.asarray(edge_index[0], np.int64)
    dst = np.asarray(edge_index[1], np.int64)

    # ---- shard edges by dst slab -------------------------------------------
    core_of = dst // NC_N
    order = np.argsort(core_of, kind="stable")
    counts = np.bincount(core_of, minlength=NCORES)
    assert counts.max() <= E_PAD, counts.max()
    starts = np.zeros(NCORES + 1, np.int64)
    starts[1:] = np.cumsum(counts)
    src_s = src[order]
    dst_s = dst[order]
    ea_s = edge_attr[order]

    e_src = np.zeros((NCORES, E_PAD), np.int64)
    e_dstl = [None] * NCORES
    eaT_sh = np.zeros((NCORES, EDIM, E_PAD), np.float32)
    for c in range(NCORES):
        n = counts[c]
        sl = slice(starts[c], starts[c + 1])
        e_src[c, :n] = src_s[sl]
        e_dstl[c] = (dst_s[sl] - c * NC_N).astype(np.int64)
        eaT_sh[c, :, :n] = ea_s[sl].T

    # ---- layer 0 (d=8): msg on CPU (tiny), node update on device -----------
    c0 = edge_attr @ Wes[0] + bes[0]
    msg0 = np.maximum(x[src] + c0, 0.0)
    agg0 = np.zeros((N, IN), np.float32)
    np.add.at(agg0, dst, msg0)
    s0 = x + agg0

    nodek8 = _get("node8", _build_node_kernel, IN)
    in_maps = [
        {
            "sT": np.ascontiguousarray(s0[c * NC_N:(c + 1) * NC_N].T),
            "W": Ws[0],
            "b": bs[0].reshape(128, 1),
        }
        for c in range(NCORES)
    ]
    res = _run(nodek8, in_maps)
    hT = np.concatenate([res[c]["hT"] for c in range(NCORES)], axis=1)  # [128, N]

    # ---- layers 1,2: msg on device, segsum on CPU, node on device ----------
    msgk = _get("msg", _build_msg_kernel, E_PAD, H)
    nodek = _get("node128", _build_node_kernel, H)
    nodeheadsk = _get("nodeheads", _build_node_heads_kernel)
    head_consts = {
        "laW1": np.asarray(inputs["laW1"], np.float32),
        "lab1": np.asarray(inputs["lab1"], np.float32).reshape(AH, 1),
        "laW2": np.asarray(inputs["laW2"], np.float32).reshape(AH, 1),
        "lcW1": np.asarray(inputs["lcW1"], np.float32),
        "lcb1": np.asarray(inputs["lcb1"], np.float32).reshape(CH, 1),
        "lcW2": np.asarray(inputs["lcW2"], np.float32).reshape(CH, 1),
        "scal": np.array(
            [[np.float32(inputs["lab2"][0]), np.float32(inputs["lcb2"][0])]],
            np.float32,
        ),
    }
    res = None
    for l in (1, 2):
        in_maps = []
        for c in range(NCORES):
            gT = np.ascontiguousarray(hT[:, e_src[c]])  # [128, E_PAD]
            in_maps.append({
                "gT": gT,
                "eaT": eaT_sh[c],
                "We": Wes[l],
                "be": bes[l].reshape(128, 1),
            })
        res = _run(msgk, in_maps)
        in_maps = []
        for c in range(NCORES):
            msgT = res[c]["msgT"]  # [128, E_PAD]
            n = counts[c]
            agg = _seg_sum_by_dst(msgT[:, :n].T, e_dstl[c], H)
            s_slab = hT[:, c * NC_N:(c + 1) * NC_N].T + agg
            m = {
                "sT": np.ascontiguousarray(s_slab.T),
                "W": Ws[l],
                "b": bs[l].reshape(128, 1),
            }
            if l == 2:
                m.update(head_consts)
            in_maps.append(m)
        res = _run(nodek if l == 1 else nodeheadsk, in_maps)
        hT = np.concatenate([res[c]["hT"] for c in range(NCORES)], axis=1)

    emb = hT.T  # [N, 128]
    loc_logits = np.concatenate(
        [res[c]["logits"][0] for c in range(NCORES)]
    ).astype(np.float32)
    loc_values = np.concatenate(
        [res[c]["values"][0] for c in range(NCORES)]
    ).astype(np.float32)

    # ---- segmented softmax / sampling glue (CPU, mirrors reference ops) ----
    import jax
    import jax.numpy as jnp

    _cpu = jax.devices("cpu")[0]

    lg = loc_logits.reshape(B, SEG)
    seg_max = lg.max(axis=1)
    shifted = (lg - seg_max[:, None]).astype(np.float32)
    ex = np.exp(shifted, dtype=np.float32)
    denom = ex.sum(axis=1, dtype=np.float32)
    logp = (shifted - np.log(denom, dtype=np.float32)[:, None]).astype(np.float32)
    p = np.exp(logp, dtype=np.float32)
    loc_entropy = (-(p * logp).sum(axis=1, dtype=np.float32)).astype(np.float32)

    with jax.default_device(_cpu):
        key = jax.random.key(42)
        u = np.asarray(
            jax.random.uniform(key, (N,), minval=1e-6, maxval=1.0 - 1e-6), np.float32
        )
    gnoise = -np.log(-np.log(u, dtype=np.float32), dtype=np.float32)
    pert = (loc_logits + gnoise).astype(np.float32).reshape(B, SEG)
    pmax = pert.max(axis=1)
    arange = np.arange(N, dtype=np.int64).reshape(B, SEG)
    cand = np.where(pert >= pmax[:, None], arange, N)
    locations = cand.min(axis=1).astype(np.int32)
    loc_log_probs = logp.reshape(-1)[locations].astype(np.float32)

    sel = emb[locations]  # [B, 128]
    maW1 = np.asarray(inputs["maW1"], np.float32)
    mab1 = np.asarray(inputs["mab1"], np.float32)
    maW2 = np.asarray(inputs["maW2"], np.float32)
    mab2 = np.asarray(inputs["mab2"], np.float32)
    mut_logits = np.maximum(sel @ maW1 + mab1, 0.0) @ maW2 + mab2
    mlg = mut_logits - mut_logits.max(axis=1, keepdims=True)
    mex = np.exp(mlg, dtype=np.float32)
    mut_logp = (mlg - np.log(mex.sum(axis=1, keepdims=True, dtype=np.float32))).astype(
        np.float32
    )
    with jax.default_device(_cpu):
        mutations = np.asarray(
            jax.random.categorical(
                jax.random.fold_in(key, 1), jnp.asarray(mut_logits), axis=-1
            ),
            np.int32,
        )
    mut_log_probs = np.take_along_axis(
        mut_logp, mutations[:, None].astype(np.int64), axis=1
    )[:, 0].astype(np.float32)
    mp = np.exp(mut_logp, dtype=np.float32)
    mut_entropy = (-(mp * mut_logp).sum(axis=1, dtype=np.float32)).astype(np.float32)

    mcW1 = np.asarray(inputs["mcW1"], np.float32)
    mcb1 = np.asarray(inputs["mcb1"], np.float32)
    mcW2 = np.asarray(inputs["mcW2"], np.float32)
    mcb2 = np.asarray(inputs["mcb2"], np.float32)
    mut_values = (np.maximum(sel @ mcW1 + mcb1, 0.0) @ mcW2 + mcb2)[:, 0].astype(
        np.float32
    )

    return (
        locations,
        np.asarray(mutations, np.int32),
        loc_log_probs,
        mut_log_probs,
        loc_entropy,
        mut_entropy,
        loc_values,
        mut_values,
        emb.astype(np.float32),
    )


# revision 22
# speedup vs baseline: 1.0845x; 1.0021x over previous
"""Trainium2 Bass kernel for nn_ActorCritic (GINE-style GNN actor-critic).

Strategy (sharding per spec hint): nodes/graphs sharded into 8 contiguous
slabs (core c owns nodes [c*32768, (c+1)*32768) == graphs [c*64, (c+1)*64));
edges sharded by the slab of their dst node so segment-sums stay local to a
slab; small weight matrices replicated.

Device (SPMD across 8 NeuronCores, one bass/Tile NEFF per stage shape):
  - MSG stage   : msgT = relu(gT + We.T @ eaT + be)      [128, E_pad] per core
  - NODE stage  : hT   = relu(W.T @ (h+agg).T + b)       [128, 32768] per core
  - HEADS stage : z=relu(laW1.T@embT+lab1); loc_logits=laW2.T@z+lab2;
                  pooled (per-graph sum); loc_values = critic MLP(pooled)
CPU does only index plumbing between stages (edge gathers h[src], per-slab
segment-sum by dst, Gumbel sampling glue) — all O(E*d) data movement, no
dense matmul work.
"""

import os
import sys

# The bass SPMD path targets the 8 axon-tunneled NeuronCores via jax's
# default platform; make sure axon wins over cpu when the host process
# hasn't pinned JAX_PLATFORMS itself.
os.environ.setdefault("JAX_PLATFORMS", "axon,cpu")

import numpy as np

sys.path.insert(0, "/opt/trn_rl_repo")

import concourse.bass as bass
import concourse.bacc as bacc
import concourse.mybir as mybir
from concourse import tile
from concourse.bass_utils import run_bass_kernel_spmd

N = 262144
E = 1048576
B = 512
IN, H, EDIM, AH, CH, NB = 8, 128, 4, 128, 128, 4
NCORES = 8
NC_N = N // NCORES          # 32768 nodes per core
NC_B = B // NCORES          # 64 graphs per core
SEG = N // B                # 512 nodes per graph
E_PAD = 135168              # per-core edge slab, padded (264 * 512)
CHUNK = 512

F32 = mybir.dt.float32

_kernel_cache = {}


def _mk_nc():
    return bacc.Bacc(
        "TRN2", target_bir_lowering=False, debug=False, num_devices=NCORES
    )


def _build_msg_kernel(e_pad, din):
    """msgT = relu(gT + We.T @ eaT + be_col)  -> [din<=128, e_pad]"""
    nc = _mk_nc()
    gT = nc.declare_dram_parameter("gT", [128, e_pad], F32, isOutput=False)
    eaT = nc.declare_dram_parameter("eaT", [EDIM, e_pad], F32, isOutput=False)
    We = nc.declare_dram_parameter("We", [EDIM, 128], F32, isOutput=False)
    be = nc.declare_dram_parameter("be", [128, 1], F32, isOutput=False)
    out = nc.declare_dram_parameter("msgT", [128, e_pad], F32, isOutput=True)

    big = 4096
    nsub = big // CHUNK
    nchunk = e_pad // big
    with tile.TileContext(nc) as tc:
        with (
            tc.tile_pool(name="const", bufs=1) as cpool,
            tc.tile_pool(name="io", bufs=3) as io,
            tc.tile_pool(name="eap", bufs=2) as eap,
            tc.tile_pool(name="sm", bufs=6) as sm,
            tc.tile_pool(name="ps", bufs=4, space="PSUM") as ps,
        ):
            We_sb = cpool.tile([EDIM, 128], F32, tag="we")
            nc.sync.dma_start(We_sb[:], We[:, :])
            be_sb = cpool.tile([128, 1], F32, tag="be")
            nc.sync.dma_start(be_sb[:], be[:, :])
            for i in range(nchunk):
                ea_t = eap.tile([EDIM, big], F32, tag="ea")
                nc.sync.dma_start(ea_t[:], eaT[:, i * big:(i + 1) * big])
                g_t = io.tile([128, big], F32, tag="g")
                nc.sync.dma_start(g_t[:], gT[:, i * big:(i + 1) * big])
                o_t = io.tile([128, big], F32, tag="o")
                for j in range(nsub):
                    sl = slice(j * CHUNK, (j + 1) * CHUNK)
                    c_ps = ps.tile([128, CHUNK], F32, tag="c")
                    nc.tensor.matmul(
                        c_ps[:], We_sb[:], ea_t[:, sl], start=True, stop=True
                    )
                    s_t = sm.tile([128, CHUNK], F32, tag="s")
                    nc.vector.tensor_add(s_t[:], g_t[:, sl], c_ps[:])
                    nc.scalar.activation(
                        o_t[:, sl], s_t[:], mybir.ActivationFunctionType.Relu,
                        bias=be_sb[:, 0:1], scale=1.0,
                    )
                nc.scalar.dma_start(out[:, i * big:(i + 1) * big], o_t[:])
    nc.compile()
    return nc


def _build_node_kernel(din):
    """hT = relu(W.T @ sT + b_col) -> [128, NC_N];  sT = (h+agg).T  [din, NC_N]"""
    nc = _mk_nc()
    sT = nc.declare_dram_parameter("sT", [din, NC_N], F32, isOutput=False)
    W = nc.declare_dram_parameter("W", [din, 128], F32, isOutput=False)
    b = nc.declare_dram_parameter("b", [128, 1], F32, isOutput=False)
    out = nc.declare_dram_parameter("hT", [128, NC_N], F32, isOutput=True)

    big = 4096
    nsub = big // CHUNK
    nchunk = NC_N // big
    with tile.TileContext(nc) as tc:
        with (
            tc.tile_pool(name="const", bufs=1) as cpool,
            tc.tile_pool(name="io", bufs=3) as io,
            tc.tile_pool(name="ps", bufs=4, space="PSUM") as ps,
        ):
            W_sb = cpool.tile([din, 128], F32, tag="w")
            nc.sync.dma_start(W_sb[:], W[:, :])
            b_sb = cpool.tile([128, 1], F32, tag="b")
            nc.sync.dma_start(b_sb[:], b[:, :])
            whole = None
            if din <= 16:  # tiny input: load all of sT once
                whole = cpool.tile([din, NC_N], F32, tag="whole")
                nc.sync.dma_start(whole[:], sT[:, :])
            for i in range(nchunk):
                if whole is None:
                    s_t = io.tile([din, big], F32, tag="s")
                    nc.sync.dma_start(s_t[:], sT[:, i * big:(i + 1) * big])
                o_t = io.tile([128, big], F32, tag="o")
                for j in range(nsub):
                    sl = slice(j * CHUNK, (j + 1) * CHUNK)
                    gsl = slice(i * big + j * CHUNK, i * big + (j + 1) * CHUNK)
                    h_ps = ps.tile([128, CHUNK], F32, tag="h")
                    nc.tensor.matmul(
                        h_ps[:], W_sb[:],
                        whole[:, gsl] if whole is not None else s_t[:, sl],
                        start=True, stop=True,
                    )
                    nc.scalar.activation(
                        o_t[:, sl], h_ps[:], mybir.ActivationFunctionType.Relu,
                        bias=b_sb[:, 0:1], scale=1.0,
                    )
                nc.scalar.dma_start(out[:, i * big:(i + 1) * big], o_t[:])
    nc.compile()
    return nc


def _build_heads_kernel():
    """loc_logits = laW2.T@relu(laW1.T@embT+lab1)+lab2  [1, NC_N]
       pooled[g]  = sum_nodes embT per graph            [128, NC_B]
       loc_values = lcW2.T@relu(lcW1.T@pooled+lcb1)+lcb2 [1, NC_B]"""
    nc = _mk_nc()
    embT = nc.declare_dram_parameter("embT", [128, NC_N], F32, isOutput=False)
    laW1 = nc.declare_dram_parameter("laW1", [128, AH], F32, isOutput=False)
    lab1 = nc.declare_dram_parameter("lab1", [AH, 1], F32, isOutput=False)
    laW2 = nc.declare_dram_parameter("laW2", [AH, 1], F32, isOutput=False)
    lcW1 = nc.declare_dram_parameter("lcW1", [128, CH], F32, isOutput=False)
    lcb1 = nc.declare_dram_parameter("lcb1", [CH, 1], F32, isOutput=False)
    lcW2 = nc.declare_dram_parameter("lcW2", [CH, 1], F32, isOutput=False)
    scal = nc.declare_dram_parameter("scal", [1, 2], F32, isOutput=False)  # lab2, lcb2
    logits = nc.declare_dram_parameter("logits", [1, NC_N], F32, isOutput=True)
    values = nc.declare_dram_parameter("values", [1, NC_B], F32, isOutput=True)

    nchunk = NC_N // SEG  # one graph per chunk
    with tile.TileContext(nc) as tc:
        with (
            tc.tile_pool(name="const", bufs=1) as cpool,
            tc.tile_pool(name="io", bufs=4) as io,
            tc.tile_pool(name="acc", bufs=1) as acc,
            tc.tile_pool(name="ps", bufs=2, space="PSUM") as ps,
        ):
            laW1_sb = cpool.tile([128, AH], F32, tag="law1")
            nc.sync.dma_start(laW1_sb[:], laW1[:, :])
            lab1_sb = cpool.tile([AH, 1], F32, tag="lab1")
            nc.sync.dma_start(lab1_sb[:], lab1[:, :])
            laW2_sb = cpool.tile([AH, 1], F32, tag="law2")
            nc.sync.dma_start(laW2_sb[:], laW2[:, :])
            lcW1_sb = cpool.tile([128, CH], F32, tag="lcw1")
            nc.sync.dma_start(lcW1_sb[:], lcW1[:, :])
            lcb1_sb = cpool.tile([CH, 1], F32, tag="lcb1")
            nc.sync.dma_start(lcb1_sb[:], lcb1[:, :])
            lcW2_sb = cpool.tile([CH, 1], F32, tag="lcw2")
            nc.sync.dma_start(lcW2_sb[:], lcW2[:, :])
            scal_sb = cpool.tile([1, 2], F32, tag="scal")
            nc.sync.dma_start(scal_sb[:], scal[:, :])

            pooled = acc.tile([128, NC_B], F32, tag="pooled")
            for i in range(nchunk):
                e_t = io.tile([128, SEG], F32, tag="e")
                nc.sync.dma_start(e_t[:], embT[:, i * SEG:(i + 1) * SEG])
                z_ps = ps.tile([AH, SEG], F32, tag="z")
                nc.tensor.matmul(z_ps[:], laW1_sb[:], e_t[:], start=True, stop=True)
                z_t = io.tile([AH, SEG], F32, tag="zs")
                nc.scalar.activation(
                    z_t[:], z_ps[:], mybir.ActivationFunctionType.Relu,
                    bias=lab1_sb[:, 0:1], scale=1.0,
                )
                l_ps = ps.tile([1, SEG], F32, tag="l")
                nc.tensor.matmul(l_ps[:], laW2_sb[:], z_t[:], start=True, stop=True)
                l_t = io.tile([1, SEG], F32, tag="ls")
                nc.scalar.activation(
                    l_t[:], l_ps[:], mybir.ActivationFunctionType.Identity,
                    bias=scal_sb[0:1, 0:1], scale=1.0,
                )
                nc.sync.dma_start(logits[:, i * SEG:(i + 1) * SEG], l_t[:])
                nc.vector.reduce_sum(
                    pooled[:, i:i + 1], e_t[:], axis=mybir.AxisListType.X
                )
            v_ps = ps.tile([CH, NC_B], F32, tag="v1")
            nc.tensor.matmul(v_ps[:], lcW1_sb[:], pooled[:], start=True, stop=True)
            v_t = io.tile([CH, NC_B], F32, tag="v1s")
            nc.scalar.activation(
                v_t[:], v_ps[:], mybir.ActivationFunctionType.Relu,
                bias=lcb1_sb[:, 0:1], scale=1.0,
            )
            v2_ps = ps.tile([1, NC_B], F32, tag="v2")
            nc.tensor.matmul(v2_ps[:], lcW2_sb[:], v_t[:], start=True, stop=True)
            v2_t = io.tile([1, NC_B], F32, tag="v2s")
            nc.scalar.activation(
                v2_t[:], v2_ps[:], mybir.ActivationFunctionType.Identity,
                bias=scal_sb[0:1, 1:2], scale=1.0,
            )
            nc.sync.dma_start(values[:, :], v2_t[:])
    nc.compile()
    return nc


def _build_node_heads_kernel():
    """Fused final GNN layer + actor/critic heads.

    hT = relu(W.T @ sT + b); loc_logits = laW2.T@relu(laW1.T@hT+lab1)+lab2;
    pooled per graph; loc_values = lcW2.T@relu(lcW1.T@pooled+lcb1)+lcb2.
    """
    nc = _mk_nc()
    sT = nc.declare_dram_parameter("sT", [H, NC_N], F32, isOutput=False)
    W = nc.declare_dram_parameter("W", [H, 128], F32, isOutput=False)
    b = nc.declare_dram_parameter("b", [128, 1], F32, isOutput=False)
    laW1 = nc.declare_dram_parameter("laW1", [128, AH], F32, isOutput=False)
    lab1 = nc.declare_dram_parameter("lab1", [AH, 1], F32, isOutput=False)
    laW2 = nc.declare_dram_parameter("laW2", [AH, 1], F32, isOutput=False)
    lcW1 = nc.declare_dram_parameter("lcW1", [128, CH], F32, isOutput=False)
    lcb1 = nc.declare_dram_parameter("lcb1", [CH, 1], F32, isOutput=False)
    lcW2 = nc.declare_dram_parameter("lcW2", [CH, 1], F32, isOutput=False)
    scal = nc.declare_dram_parameter("scal", [1, 2], F32, isOutput=False)
    hT_o = nc.declare_dram_parameter("hT", [128, NC_N], F32, isOutput=True)
    logits = nc.declare_dram_parameter("logits", [1, NC_N], F32, isOutput=True)
    values = nc.declare_dram_parameter("values", [1, NC_B], F32, isOutput=True)

    nchunk = NC_N // SEG
    with tile.TileContext(nc) as tc:
        with (
            tc.tile_pool(name="const", bufs=1) as cpool,
            tc.tile_pool(name="io", bufs=4) as io,
            tc.tile_pool(name="acc", bufs=1) as acc,
            tc.tile_pool(name="psA", bufs=2, space="PSUM") as psA,
            tc.tile_pool(name="psB", bufs=2, space="PSUM") as psB,
            tc.tile_pool(name="psC", bufs=1, space="PSUM") as psC,
        ):
            def ld(name, shape):
                t = cpool.tile(shape, F32, tag=name)
                nc.sync.dma_start(t[:], {
                    "w": W, "b": b, "law1": laW1, "lab1": lab1, "law2": laW2,
                    "lcw1": lcW1, "lcb1": lcb1, "lcw2": lcW2, "scal": scal,
                }[name][:, :])
                return t

            W_sb = ld("w", [H, 128])
            b_sb = ld("b", [128, 1])
            laW1_sb = ld("law1", [128, AH])
            lab1_sb = ld("lab1", [AH, 1])
            laW2_sb = ld("law2", [AH, 1])
            lcW1_sb = ld("lcw1", [128, CH])
            lcb1_sb = ld("lcb1", [CH, 1])
            lcW2_sb = ld("lcw2", [CH, 1])
            scal_sb = ld("scal", [1, 2])

            pooled = acc.tile([128, NC_B], F32, tag="pooled")
            big = 4096
            nsub = big // SEG
            for i in range(NC_N // big):
                s_t = io.tile([H, big], F32, tag="s")
                nc.sync.dma_start(s_t[:], sT[:, i * big:(i + 1) * big])
                h_t = io.tile([128, big], F32, tag="h_sb")
                l_t = io.tile([1, big], F32, tag="l_sb")
                for j in range(nsub):
                    sl = slice(j * SEG, (j + 1) * SEG)
                    h_ps = psA.tile([128, SEG], F32, tag="h")
                    nc.tensor.matmul(
                        h_ps[:], W_sb[:], s_t[:, sl], start=True, stop=True
                    )
                    nc.scalar.activation(
                        h_t[:, sl], h_ps[:], mybir.ActivationFunctionType.Relu,
                        bias=b_sb[:, 0:1], scale=1.0,
                    )
                    z_ps = psA.tile([AH, SEG], F32, tag="z")
                    nc.tensor.matmul(
                        z_ps[:], laW1_sb[:], h_t[:, sl], start=True, stop=True
                    )
                    z_t = io.tile([AH, SEG], F32, tag="z_sb")
                    nc.scalar.activation(
                        z_t[:], z_ps[:], mybir.ActivationFunctionType.Relu,
                        bias=lab1_sb[:, 0:1], scale=1.0,
                    )
                    l_ps = psB.tile([1, SEG], F32, tag="l")
                    nc.tensor.matmul(
                        l_ps[:], laW2_sb[:], z_t[:], start=True, stop=True
                    )
                    nc.scalar.activation(
                        l_t[:, sl], l_ps[:], mybir.ActivationFunctionType.Identity,
                        bias=scal_sb[0:1, 0:1], scale=1.0,
                    )
                    nc.vector.reduce_sum(
                        pooled[:, i * nsub + j:i * nsub + j + 1], h_t[:, sl],
                        axis=mybir.AxisListType.X,
                    )
                nc.scalar.dma_start(hT_o[:, i * big:(i + 1) * big], h_t[:])
                nc.scalar.dma_start(logits[:, i * big:(i + 1) * big], l_t[:])
            v_ps = psC.tile([CH, NC_B], F32, tag="v1")
            nc.tensor.matmul(v_ps[:], lcW1_sb[:], pooled[:], start=True, stop=True)
            v_t = io.tile([CH, NC_B], F32, tag="v1s")
            nc.scalar.activation(
                v_t[:], v_ps[:], mybir.ActivationFunctionType.Relu,
                bias=lcb1_sb[:, 0:1], scale=1.0,
            )
            v2_ps = psC.tile([1, NC_B], F32, tag="v2")
            nc.tensor.matmul(v2_ps[:], lcW2_sb[:], v_t[:], start=True, stop=True)
            v2_t = io.tile([1, NC_B], F32, tag="v2s")
            nc.scalar.activation(
                v2_t[:], v2_ps[:], mybir.ActivationFunctionType.Identity,
                bias=scal_sb[0:1, 1:2], scale=1.0,
            )
            nc.sync.dma_start(values[:, :], v2_t[:])
    nc.compile()
    return nc


def _get(name, builder, *args):
    if name not in _kernel_cache:
        _kernel_cache[name] = builder(*args)
    return _kernel_cache[name]


_EXEC_NS = []
_TRACE = os.environ.get("BASS_KERNEL_TRACE", "0") == "1"


def _run(nc, in_maps):
    if _TRACE:
        try:
            res = run_bass_kernel_spmd(
                nc, in_maps, list(range(NCORES)), trace=True
            )
            if res.exec_time_ns:
                _EXEC_NS.append(res.exec_time_ns)
            return res.results
        except Exception as e:  # trace infra unavailable — fall back untraced
            print(f"trace failed ({type(e).__name__}: {e}); untraced rerun",
                  file=sys.stderr)
    res = run_bass_kernel_spmd(nc, in_maps, list(range(NCORES)))
    if res.exec_time_ns:
        _EXEC_NS.append(res.exec_time_ns)
    return res.results


def _seg_sum_by_dst(msg, dst_local, ncol):
    out = np.zeros((NC_N, ncol), dtype=np.float32)
    np.add.at(out, dst_local, msg)
    return out


def kernel(**inputs):
    x = np.asarray(inputs["x"], np.float32)
    edge_index = np.asarray(inputs["edge_index"])
    edge_attr = np.asarray(inputs["edge_attr"], np.float32)

    Wes = [np.asarray(inputs[f"We{l}"], np.float32) for l in range(3)]
    bes = [np.asarray(inputs[f"be{l}"], np.float32) for l in range(3)]
    Ws = [np.asarray(inputs[f"W{l}"], np.float32) for l in range(3)]
    bs = [np.asarray(inputs[f"b{l}"], np.float32) for l in range(3)]

    src = np.asarray(edge_index[0], np.int64)
    dst = np.asarray(edge_index[1], np.int64)

    # ---- shard edges by dst slab -------------------------------------------
    core_of = dst // NC_N
    order = np.argsort(core_of, kind="stable")
    counts = np.bincount(core_of, minlength=NCORES)
    assert counts.max() <= E_PAD, counts.max()
    starts = np.zeros(NCORES + 1, np.int64)
    starts[1:] = np.cumsum(counts)
    src_s = src[order]
    dst_s = dst[order]
    ea_s = edge_attr[order]

    e_src = np.zeros((NCORES, E_PAD), np.int64)
    e_dstl = [None] * NCORES
    eaT_sh = np.zeros((NCORES, EDIM, E_PAD), np.float32)
    for c in range(NCORES):
        n = counts[c]
        sl = slice(starts[c], starts[c + 1])
        e_src[c, :n] = src_s[sl]
        e_dstl[c] = (dst_s[sl] - c * NC_N).astype(np.int64)
        eaT_sh[c, :, :n] = ea_s[sl].T

    # ---- layer 0 (d=8): msg on CPU (tiny), node update on device -----------
    c0 = edge_attr @ Wes[0] + bes[0]
    msg0 = np.maximum(x[src] + c0, 0.0)
    agg0 = np.zeros((N, IN), np.float32)
    np.add.at(agg0, dst, msg0)
    s0 = x + agg0

    nodek8 = _get("node8", _build_node_kernel, IN)
    in_maps = [
        {
            "sT": np.ascontiguousarray(s0[c * NC_N:(c + 1) * NC_N].T),
            "W": Ws[0],
            "b": bs[0].reshape(128, 1),
        }
        for c in range(NCORES)
    ]
    res = _run(nodek8, in_maps)
    hT = np.concatenate([res[c]["hT"] for c in range(NCORES)], axis=1)  # [128, N]

    # ---- layers 1,2: msg on device, segsum on CPU, node on device ----------
    msgk = _get("msg", _build_msg_kernel, E_PAD, H)
    nodek = _get("node128", _build_node_kernel, H)
    nodeheadsk = _get("nodeheads", _build_node_heads_kernel)
    head_consts = {
        "laW1": np.asarray(inputs["laW1"], np.float32),
        "lab1": np.asarray(inputs["lab1"], np.float32).reshape(AH, 1),
        "laW2": np.asarray(inputs["laW2"], np.float32).reshape(AH, 1),
        "lcW1": np.asarray(inputs["lcW1"], np.float32),
        "lcb1": np.asarray(inputs["lcb1"], np.float32).reshape(CH, 1),
        "lcW2": np.asarray(inputs["lcW2"], np.float32).reshape(CH, 1),
        "scal": np.array(
            [[np.float32(inputs["lab2"][0]), np.float32(inputs["lcb2"][0])]],
            np.float32,
        ),
    }
    res = None
    for l in (1, 2):
        in_maps = []
        for c in range(NCORES):
            gT = np.ascontiguousarray(hT[:, e_src[c]])  # [128, E_PAD]
            in_maps.append({
                "gT": gT,
                "eaT": eaT_sh[c],
                "We": Wes[l],
                "be": bes[l].reshape(128, 1),
            })
        res = _run(msgk, in_maps)
        in_maps = []
        for c in range(NCORES):
            msgT = res[c]["msgT"]  # [128, E_PAD]
            n = counts[c]
            agg = _seg_sum_by_dst(msgT[:, :n].T, e_dstl[c], H)
            s_slab = hT[:, c * NC_N:(c + 1) * NC_N].T + agg
            m = {
                "sT": np.ascontiguousarray(s_slab.T),
                "W": Ws[l],
                "b": bs[l].reshape(128, 1),
            }
            if l == 2:
                m.update(head_consts)
            in_maps.append(m)
        res = _run(nodek if l == 1 else nodeheadsk, in_maps)
        hT = np.concatenate([res[c]["hT"] for c in range(NCORES)], axis=1)

    emb = hT.T  # [N, 128]
    loc_logits = np.concatenate(
        [res[c]["logits"][0] for c in range(NCORES)]
    ).astype(np.float32)
    loc_values = np.concatenate(
        [res[c]["values"][0] for c in range(NCORES)]
    ).astype(np.float32)

    # ---- segmented softmax / sampling glue (CPU, mirrors reference ops) ----
    import jax
    import jax.numpy as jnp

    _cpu = jax.devices("cpu")[0]

    lg = loc_logits.reshape(B, SEG)
    seg_max = lg.max(axis=1)
    shifted = (lg - seg_max[:, None]).astype(np.float32)
    ex = np.exp(shifted, dtype=np.float32)
    denom = ex.sum(axis=1, dtype=np.float32)
    logp = (shifted - np.log(denom, dtype=np.float32)[:, None]).astype(np.float32)
    p = np.exp(logp, dtype=np.float32)
    loc_entropy = (-(p * logp).sum(axis=1, dtype=np.float32)).astype(np.float32)

    with jax.default_device(_cpu):
        key = jax.random.key(42)
        u = np.asarray(
            jax.random.uniform(key, (N,), minval=1e-6, maxval=1.0 - 1e-6), np.float32
        )
    gnoise = -np.log(-np.log(u, dtype=np.float32), dtype=np.float32)
    pert = (loc_logits + gnoise).astype(np.float32).reshape(B, SEG)
    pmax = pert.max(axis=1)
    arange = np.arange(N, dtype=np.int64).reshape(B, SEG)
    cand = np.where(pert >= pmax[:, None], arange, N)
    locations = cand.min(axis=1).astype(np.int32)
    loc_log_probs = logp.reshape(-1)[locations].astype(np.float32)

    sel = emb[locations]  # [B, 128]
    maW1 = np.asarray(inputs["maW1"], np.float32)
    mab1 = np.asarray(inputs["mab1"], np.float32)
    maW2 = np.asarray(inputs["maW2"], np.float32)
    mab2 = np.asarray(inputs["mab2"], np.float32)
    mut_logits = np.maximum(sel @ maW1 + mab1, 0.0) @ maW2 + mab2
    mlg = mut_logits - mut_logits.max(axis=1, keepdims=True)
    mex = np.exp(mlg, dtype=np.float32)
    mut_logp = (mlg - np.log(mex.sum(axis=1, keepdims=True, dtype=np.float32))).astype(
        np.float32
    )
    with jax.default_device(_cpu):
        mutations = np.asarray(
            jax.random.categorical(
                jax.random.fold_in(key, 1), jnp.asarray(mut_logits), axis=-1
            ),
            np.int32,
        )
    mut_log_probs = np.take_along_axis(
        mut_logp, mutations[:, None].astype(np.int64), axis=1
    )[:, 0].astype(np.float32)
    mp = np.exp(mut_logp, dtype=np.float32)
    mut_entropy = (-(mp * mut_logp).sum(axis=1, dtype=np.float32)).astype(np.float32)

    mcW1 = np.asarray(inputs["mcW1"], np.float32)
    mcb1 = np.asarray(inputs["mcb1"], np.float32)
    mcW2 = np.asarray(inputs["mcW2"], np.float32)
    mcb2 = np.asarray(inputs["mcb2"], np.float32)
    mut_values = (np.maximum(sel @ mcW1 + mcb1, 0.0) @ mcW2 + mcb2)[:, 0].astype(
        np.float32
    )

    return (
        locations,
        np.asarray(mutations, np.int32),
        loc_log_probs,
        mut_log_probs,
        loc_entropy,
        mut_entropy,
        loc_values,
        mut_values,
        emb.astype(np.float32),
    )


# revision 23
# speedup vs baseline: 1.0989x; 1.0132x over previous
"""Trainium2 Bass kernel for nn_ActorCritic (GINE-style GNN actor-critic).

Strategy (sharding per spec hint): nodes/graphs sharded into 8 contiguous
slabs (core c owns nodes [c*32768, (c+1)*32768) == graphs [c*64, (c+1)*64));
edges sharded by the slab of their dst node so segment-sums stay local to a
slab; small weight matrices replicated.

Device (SPMD across 8 NeuronCores, one bass/Tile NEFF per stage shape):
  - MSG stage   : msgT = relu(gT + We.T @ eaT + be)      [128, E_pad] per core
  - NODE stage  : hT   = relu(W.T @ (h+agg).T + b)       [128, 32768] per core
  - HEADS stage : z=relu(laW1.T@embT+lab1); loc_logits=laW2.T@z+lab2;
                  pooled (per-graph sum); loc_values = critic MLP(pooled)
CPU does only index plumbing between stages (edge gathers h[src], per-slab
segment-sum by dst, Gumbel sampling glue) — all O(E*d) data movement, no
dense matmul work.
"""

import os
import sys

# The bass SPMD path targets the 8 axon-tunneled NeuronCores via jax's
# default platform; make sure axon wins over cpu when the host process
# hasn't pinned JAX_PLATFORMS itself.
os.environ.setdefault("JAX_PLATFORMS", "axon,cpu")

import numpy as np

sys.path.insert(0, "/opt/trn_rl_repo")

import concourse.bass as bass
import concourse.bacc as bacc
import concourse.mybir as mybir
from concourse import tile
from concourse.bass_utils import run_bass_kernel_spmd

N = 262144
E = 1048576
B = 512
IN, H, EDIM, AH, CH, NB = 8, 128, 4, 128, 128, 4
NCORES = 8
NC_N = N // NCORES          # 32768 nodes per core
NC_B = B // NCORES          # 64 graphs per core
SEG = N // B                # 512 nodes per graph
E_PAD = 135168              # per-core edge slab, padded (264 * 512)
CHUNK = 512

F32 = mybir.dt.float32

_kernel_cache = {}


def _mk_nc():
    return bacc.Bacc(
        "TRN2", target_bir_lowering=False, debug=False, num_devices=NCORES
    )


def _build_msg_kernel(e_pad, din):
    """msgT = relu(gT + We.T @ eaT + be_col)  -> [din<=128, e_pad]"""
    nc = _mk_nc()
    gT = nc.declare_dram_parameter("gT", [128, e_pad], F32, isOutput=False)
    eaT = nc.declare_dram_parameter("eaT", [EDIM, e_pad], F32, isOutput=False)
    We = nc.declare_dram_parameter("We", [EDIM, 128], F32, isOutput=False)
    be = nc.declare_dram_parameter("be", [128, 1], F32, isOutput=False)
    out = nc.declare_dram_parameter("msgT", [128, e_pad], F32, isOutput=True)

    big = 4096
    nsub = big // CHUNK
    nchunk = e_pad // big
    with tile.TileContext(nc) as tc:
        with (
            tc.tile_pool(name="const", bufs=1) as cpool,
            tc.tile_pool(name="io", bufs=3) as io,
            tc.tile_pool(name="eap", bufs=2) as eap,
            tc.tile_pool(name="sm", bufs=6) as sm,
            tc.tile_pool(name="ps", bufs=4, space="PSUM") as ps,
        ):
            We_sb = cpool.tile([EDIM, 128], F32, tag="we")
            nc.sync.dma_start(We_sb[:], We[:, :])
            be_sb = cpool.tile([128, 1], F32, tag="be")
            nc.sync.dma_start(be_sb[:], be[:, :])
            for i in range(nchunk):
                ea_t = eap.tile([EDIM, big], F32, tag="ea")
                nc.sync.dma_start(ea_t[:], eaT[:, i * big:(i + 1) * big])
                g_t = io.tile([128, big], F32, tag="g")
                nc.sync.dma_start(g_t[:], gT[:, i * big:(i + 1) * big])
                o_t = io.tile([128, big], F32, tag="o")
                for j in range(nsub):
                    sl = slice(j * CHUNK, (j + 1) * CHUNK)
                    c_ps = ps.tile([128, CHUNK], F32, tag="c")
                    nc.tensor.matmul(
                        c_ps[:], We_sb[:], ea_t[:, sl], start=True, stop=True
                    )
                    s_t = sm.tile([128, CHUNK], F32, tag="s")
                    nc.vector.tensor_add(s_t[:], g_t[:, sl], c_ps[:])
                    nc.scalar.activation(
                        o_t[:, sl], s_t[:], mybir.ActivationFunctionType.Relu,
                        bias=be_sb[:, 0:1], scale=1.0,
                    )
                nc.scalar.dma_start(out[:, i * big:(i + 1) * big], o_t[:])
    nc.compile()
    return nc


def _build_node_kernel(din):
    """hT = relu(W.T @ sT + b_col) -> [128, NC_N];  sT = (h+agg).T  [din, NC_N]"""
    nc = _mk_nc()
    sT = nc.declare_dram_parameter("sT", [din, NC_N], F32, isOutput=False)
    W = nc.declare_dram_parameter("W", [din, 128], F32, isOutput=False)
    b = nc.declare_dram_parameter("b", [128, 1], F32, isOutput=False)
    out = nc.declare_dram_parameter("hT", [128, NC_N], F32, isOutput=True)

    big = 4096
    nsub = big // CHUNK
    nchunk = NC_N // big
    with tile.TileContext(nc) as tc:
        with (
            tc.tile_pool(name="const", bufs=1) as cpool,
            tc.tile_pool(name="io", bufs=3) as io,
            tc.tile_pool(name="ps", bufs=8, space="PSUM") as ps,
        ):
            W_sb = cpool.tile([din, 128], F32, tag="w")
            nc.sync.dma_start(W_sb[:], W[:, :])
            b_sb = cpool.tile([128, 1], F32, tag="b")
            nc.sync.dma_start(b_sb[:], b[:, :])
            whole = None
            if din <= 16:  # tiny input: load all of sT once, in quarters
                whole = cpool.tile([din, NC_N], F32, tag="whole")
                q = NC_N // 4
                for k in range(4):
                    nc.sync.dma_start(
                        whole[:, k * q:(k + 1) * q], sT[:, k * q:(k + 1) * q]
                    )
            for i in range(nchunk):
                if whole is None:
                    s_t = io.tile([din, big], F32, tag="s")
                    nc.sync.dma_start(s_t[:], sT[:, i * big:(i + 1) * big])
                o_t = io.tile([128, big], F32, tag="o")
                for j in range(nsub):
                    sl = slice(j * CHUNK, (j + 1) * CHUNK)
                    gsl = slice(i * big + j * CHUNK, i * big + (j + 1) * CHUNK)
                    h_ps = ps.tile([128, CHUNK], F32, tag="h")
                    nc.tensor.matmul(
                        h_ps[:], W_sb[:],
                        whole[:, gsl] if whole is not None else s_t[:, sl],
                        start=True, stop=True,
                    )
                    nc.scalar.activation(
                        o_t[:, sl], h_ps[:], mybir.ActivationFunctionType.Relu,
                        bias=b_sb[:, 0:1], scale=1.0,
                    )
                nc.scalar.dma_start(out[:, i * big:(i + 1) * big], o_t[:])
    nc.compile()
    return nc


def _build_heads_kernel():
    """loc_logits = laW2.T@relu(laW1.T@embT+lab1)+lab2  [1, NC_N]
       pooled[g]  = sum_nodes embT per graph            [128, NC_B]
       loc_values = lcW2.T@relu(lcW1.T@pooled+lcb1)+lcb2 [1, NC_B]"""
    nc = _mk_nc()
    embT = nc.declare_dram_parameter("embT", [128, NC_N], F32, isOutput=False)
    laW1 = nc.declare_dram_parameter("laW1", [128, AH], F32, isOutput=False)
    lab1 = nc.declare_dram_parameter("lab1", [AH, 1], F32, isOutput=False)
    laW2 = nc.declare_dram_parameter("laW2", [AH, 1], F32, isOutput=False)
    lcW1 = nc.declare_dram_parameter("lcW1", [128, CH], F32, isOutput=False)
    lcb1 = nc.declare_dram_parameter("lcb1", [CH, 1], F32, isOutput=False)
    lcW2 = nc.declare_dram_parameter("lcW2", [CH, 1], F32, isOutput=False)
    scal = nc.declare_dram_parameter("scal", [1, 2], F32, isOutput=False)  # lab2, lcb2
    logits = nc.declare_dram_parameter("logits", [1, NC_N], F32, isOutput=True)
    values = nc.declare_dram_parameter("values", [1, NC_B], F32, isOutput=True)

    nchunk = NC_N // SEG  # one graph per chunk
    with tile.TileContext(nc) as tc:
        with (
            tc.tile_pool(name="const", bufs=1) as cpool,
            tc.tile_pool(name="io", bufs=4) as io,
            tc.tile_pool(name="acc", bufs=1) as acc,
            tc.tile_pool(name="ps", bufs=2, space="PSUM") as ps,
        ):
            laW1_sb = cpool.tile([128, AH], F32, tag="law1")
            nc.sync.dma_start(laW1_sb[:], laW1[:, :])
            lab1_sb = cpool.tile([AH, 1], F32, tag="lab1")
            nc.sync.dma_start(lab1_sb[:], lab1[:, :])
            laW2_sb = cpool.tile([AH, 1], F32, tag="law2")
            nc.sync.dma_start(laW2_sb[:], laW2[:, :])
            lcW1_sb = cpool.tile([128, CH], F32, tag="lcw1")
            nc.sync.dma_start(lcW1_sb[:], lcW1[:, :])
            lcb1_sb = cpool.tile([CH, 1], F32, tag="lcb1")
            nc.sync.dma_start(lcb1_sb[:], lcb1[:, :])
            lcW2_sb = cpool.tile([CH, 1], F32, tag="lcw2")
            nc.sync.dma_start(lcW2_sb[:], lcW2[:, :])
            scal_sb = cpool.tile([1, 2], F32, tag="scal")
            nc.sync.dma_start(scal_sb[:], scal[:, :])

            pooled = acc.tile([128, NC_B], F32, tag="pooled")
            for i in range(nchunk):
                e_t = io.tile([128, SEG], F32, tag="e")
                nc.sync.dma_start(e_t[:], embT[:, i * SEG:(i + 1) * SEG])
                z_ps = ps.tile([AH, SEG], F32, tag="z")
                nc.tensor.matmul(z_ps[:], laW1_sb[:], e_t[:], start=True, stop=True)
                z_t = io.tile([AH, SEG], F32, tag="zs")
                nc.scalar.activation(
                    z_t[:], z_ps[:], mybir.ActivationFunctionType.Relu,
                    bias=lab1_sb[:, 0:1], scale=1.0,
                )
                l_ps = ps.tile([1, SEG], F32, tag="l")
                nc.tensor.matmul(l_ps[:], laW2_sb[:], z_t[:], start=True, stop=True)
                l_t = io.tile([1, SEG], F32, tag="ls")
                nc.scalar.activation(
                    l_t[:], l_ps[:], mybir.ActivationFunctionType.Identity,
                    bias=scal_sb[0:1, 0:1], scale=1.0,
                )
                nc.sync.dma_start(logits[:, i * SEG:(i + 1) * SEG], l_t[:])
                nc.vector.reduce_sum(
                    pooled[:, i:i + 1], e_t[:], axis=mybir.AxisListType.X
                )
            v_ps = ps.tile([CH, NC_B], F32, tag="v1")
            nc.tensor.matmul(v_ps[:], lcW1_sb[:], pooled[:], start=True, stop=True)
            v_t = io.tile([CH, NC_B], F32, tag="v1s")
            nc.scalar.activation(
                v_t[:], v_ps[:], mybir.ActivationFunctionType.Relu,
                bias=lcb1_sb[:, 0:1], scale=1.0,
            )
            v2_ps = ps.tile([1, NC_B], F32, tag="v2")
            nc.tensor.matmul(v2_ps[:], lcW2_sb[:], v_t[:], start=True, stop=True)
            v2_t = io.tile([1, NC_B], F32, tag="v2s")
            nc.scalar.activation(
                v2_t[:], v2_ps[:], mybir.ActivationFunctionType.Identity,
                bias=scal_sb[0:1, 1:2], scale=1.0,
            )
            nc.sync.dma_start(values[:, :], v2_t[:])
    nc.compile()
    return nc


def _build_node_heads_kernel():
    """Fused final GNN layer + actor/critic heads.

    hT = relu(W.T @ sT + b); loc_logits = laW2.T@relu(laW1.T@hT+lab1)+lab2;
    pooled per graph; loc_values = lcW2.T@relu(lcW1.T@pooled+lcb1)+lcb2.
    """
    nc = _mk_nc()
    sT = nc.declare_dram_parameter("sT", [H, NC_N], F32, isOutput=False)
    W = nc.declare_dram_parameter("W", [H, 128], F32, isOutput=False)
    b = nc.declare_dram_parameter("b", [128, 1], F32, isOutput=False)
    laW1 = nc.declare_dram_parameter("laW1", [128, AH], F32, isOutput=False)
    lab1 = nc.declare_dram_parameter("lab1", [AH, 1], F32, isOutput=False)
    laW2 = nc.declare_dram_parameter("laW2", [AH, 1], F32, isOutput=False)
    lcW1 = nc.declare_dram_parameter("lcW1", [128, CH], F32, isOutput=False)
    lcb1 = nc.declare_dram_parameter("lcb1", [CH, 1], F32, isOutput=False)
    lcW2 = nc.declare_dram_parameter("lcW2", [CH, 1], F32, isOutput=False)
    scal = nc.declare_dram_parameter("scal", [1, 2], F32, isOutput=False)
    hT_o = nc.declare_dram_parameter("hT", [128, NC_N], F32, isOutput=True)
    logits = nc.declare_dram_parameter("logits", [1, NC_N], F32, isOutput=True)
    values = nc.declare_dram_parameter("values", [1, NC_B], F32, isOutput=True)

    nchunk = NC_N // SEG
    with tile.TileContext(nc) as tc:
        with (
            tc.tile_pool(name="const", bufs=1) as cpool,
            tc.tile_pool(name="io", bufs=4) as io,
            tc.tile_pool(name="acc", bufs=1) as acc,
            tc.tile_pool(name="psA", bufs=2, space="PSUM") as psA,
            tc.tile_pool(name="psB", bufs=2, space="PSUM") as psB,
            tc.tile_pool(name="psC", bufs=1, space="PSUM") as psC,
        ):
            def ld(name, shape):
                t = cpool.tile(shape, F32, tag=name)
                nc.sync.dma_start(t[:], {
                    "w": W, "b": b, "law1": laW1, "lab1": lab1, "law2": laW2,
                    "lcw1": lcW1, "lcb1": lcb1, "lcw2": lcW2, "scal": scal,
                }[name][:, :])
                return t

            W_sb = ld("w", [H, 128])
            b_sb = ld("b", [128, 1])
            laW1_sb = ld("law1", [128, AH])
            lab1_sb = ld("lab1", [AH, 1])
            laW2_sb = ld("law2", [AH, 1])
            lcW1_sb = ld("lcw1", [128, CH])
            lcb1_sb = ld("lcb1", [CH, 1])
            lcW2_sb = ld("lcw2", [CH, 1])
            scal_sb = ld("scal", [1, 2])

            pooled = acc.tile([128, NC_B], F32, tag="pooled")
            big = 4096
            nsub = big // SEG
            for i in range(NC_N // big):
                s_t = io.tile([H, big], F32, tag="s")
                nc.sync.dma_start(s_t[:], sT[:, i * big:(i + 1) * big])
                h_t = io.tile([128, big], F32, tag="h_sb")
                l_t = io.tile([1, big], F32, tag="l_sb")
                for j in range(nsub):
                    sl = slice(j * SEG, (j + 1) * SEG)
                    h_ps = psA.tile([128, SEG], F32, tag="h")
                    nc.tensor.matmul(
                        h_ps[:], W_sb[:], s_t[:, sl], start=True, stop=True
                    )
                    nc.scalar.activation(
                        h_t[:, sl], h_ps[:], mybir.ActivationFunctionType.Relu,
                        bias=b_sb[:, 0:1], scale=1.0,
                    )
                    z_ps = psA.tile([AH, SEG], F32, tag="z")
                    nc.tensor.matmul(
                        z_ps[:], laW1_sb[:], h_t[:, sl], start=True, stop=True
                    )
                    z_t = io.tile([AH, SEG], F32, tag="z_sb")
                    nc.scalar.activation(
                        z_t[:], z_ps[:], mybir.ActivationFunctionType.Relu,
                        bias=lab1_sb[:, 0:1], scale=1.0,
                    )
                    l_ps = psB.tile([1, SEG], F32, tag="l")
                    nc.tensor.matmul(
                        l_ps[:], laW2_sb[:], z_t[:], start=True, stop=True
                    )
                    nc.scalar.activation(
                        l_t[:, sl], l_ps[:], mybir.ActivationFunctionType.Identity,
                        bias=scal_sb[0:1, 0:1], scale=1.0,
                    )
                    nc.vector.reduce_sum(
                        pooled[:, i * nsub + j:i * nsub + j + 1], h_t[:, sl],
                        axis=mybir.AxisListType.X,
                    )
                nc.scalar.dma_start(hT_o[:, i * big:(i + 1) * big], h_t[:])
                nc.scalar.dma_start(logits[:, i * big:(i + 1) * big], l_t[:])
            v_ps = psC.tile([CH, NC_B], F32, tag="v1")
            nc.tensor.matmul(v_ps[:], lcW1_sb[:], pooled[:], start=True, stop=True)
            v_t = io.tile([CH, NC_B], F32, tag="v1s")
            nc.scalar.activation(
                v_t[:], v_ps[:], mybir.ActivationFunctionType.Relu,
                bias=lcb1_sb[:, 0:1], scale=1.0,
            )
            v2_ps = psC.tile([1, NC_B], F32, tag="v2")
            nc.tensor.matmul(v2_ps[:], lcW2_sb[:], v_t[:], start=True, stop=True)
            v2_t = io.tile([1, NC_B], F32, tag="v2s")
            nc.scalar.activation(
                v2_t[:], v2_ps[:], mybir.ActivationFunctionType.Identity,
                bias=scal_sb[0:1, 1:2], scale=1.0,
            )
            nc.sync.dma_start(values[:, :], v2_t[:])
    nc.compile()
    return nc


def _get(name, builder, *args):
    if name not in _kernel_cache:
        _kernel_cache[name] = builder(*args)
    return _kernel_cache[name]


_EXEC_NS = []
_TRACE = os.environ.get("BASS_KERNEL_TRACE", "0") == "1"


def _run(nc, in_maps):
    if _TRACE:
        try:
            res = run_bass_kernel_spmd(
                nc, in_maps, list(range(NCORES)), trace=True
            )
            if res.exec_time_ns:
                _EXEC_NS.append(res.exec_time_ns)
            return res.results
        except Exception as e:  # trace infra unavailable — fall back untraced
            print(f"trace failed ({type(e).__name__}: {e}); untraced rerun",
                  file=sys.stderr)
    res = run_bass_kernel_spmd(nc, in_maps, list(range(NCORES)))
    if res.exec_time_ns:
        _EXEC_NS.append(res.exec_time_ns)
    return res.results


def _seg_sum_by_dst(msg, dst_local, ncol):
    out = np.zeros((NC_N, ncol), dtype=np.float32)
    np.add.at(out, dst_local, msg)
    return out


def kernel(**inputs):
    x = np.asarray(inputs["x"], np.float32)
    edge_index = np.asarray(inputs["edge_index"])
    edge_attr = np.asarray(inputs["edge_attr"], np.float32)

    Wes = [np.asarray(inputs[f"We{l}"], np.float32) for l in range(3)]
    bes = [np.asarray(inputs[f"be{l}"], np.float32) for l in range(3)]
    Ws = [np.asarray(inputs[f"W{l}"], np.float32) for l in range(3)]
    bs = [np.asarray(inputs[f"b{l}"], np.float32) for l in range(3)]

    src = np.asarray(edge_index[0], np.int64)
    dst = np.asarray(edge_index[1], np.int64)

    # ---- shard edges by dst slab -------------------------------------------
    core_of = dst // NC_N
    order = np.argsort(core_of, kind="stable")
    counts = np.bincount(core_of, minlength=NCORES)
    assert counts.max() <= E_PAD, counts.max()
    starts = np.zeros(NCORES + 1, np.int64)
    starts[1:] = np.cumsum(counts)
    src_s = src[order]
    dst_s = dst[order]
    ea_s = edge_attr[order]

    e_src = np.zeros((NCORES, E_PAD), np.int64)
    e_dstl = [None] * NCORES
    eaT_sh = np.zeros((NCORES, EDIM, E_PAD), np.float32)
    for c in range(NCORES):
        n = counts[c]
        sl = slice(starts[c], starts[c + 1])
        e_src[c, :n] = src_s[sl]
        e_dstl[c] = (dst_s[sl] - c * NC_N).astype(np.int64)
        eaT_sh[c, :, :n] = ea_s[sl].T

    # ---- layer 0 (d=8): msg on CPU (tiny), node update on device -----------
    c0 = edge_attr @ Wes[0] + bes[0]
    msg0 = np.maximum(x[src] + c0, 0.0)
    agg0 = np.zeros((N, IN), np.float32)
    np.add.at(agg0, dst, msg0)
    s0 = x + agg0

    nodek8 = _get("node8", _build_node_kernel, IN)
    in_maps = [
        {
            "sT": np.ascontiguousarray(s0[c * NC_N:(c + 1) * NC_N].T),
            "W": Ws[0],
            "b": bs[0].reshape(128, 1),
        }
        for c in range(NCORES)
    ]
    res = _run(nodek8, in_maps)
    hT = np.concatenate([res[c]["hT"] for c in range(NCORES)], axis=1)  # [128, N]

    # ---- layers 1,2: msg on device, segsum on CPU, node on device ----------
    msgk = _get("msg", _build_msg_kernel, E_PAD, H)
    nodek = _get("node128", _build_node_kernel, H)
    nodeheadsk = _get("nodeheads", _build_node_heads_kernel)
    head_consts = {
        "laW1": np.asarray(inputs["laW1"], np.float32),
        "lab1": np.asarray(inputs["lab1"], np.float32).reshape(AH, 1),
        "laW2": np.asarray(inputs["laW2"], np.float32).reshape(AH, 1),
        "lcW1": np.asarray(inputs["lcW1"], np.float32),
        "lcb1": np.asarray(inputs["lcb1"], np.float32).reshape(CH, 1),
        "lcW2": np.asarray(inputs["lcW2"], np.float32).reshape(CH, 1),
        "scal": np.array(
            [[np.float32(inputs["lab2"][0]), np.float32(inputs["lcb2"][0])]],
            np.float32,
        ),
    }
    res = None
    for l in (1, 2):
        in_maps = []
        for c in range(NCORES):
            gT = np.ascontiguousarray(hT[:, e_src[c]])  # [128, E_PAD]
            in_maps.append({
                "gT": gT,
                "eaT": eaT_sh[c],
                "We": Wes[l],
                "be": bes[l].reshape(128, 1),
            })
        res = _run(msgk, in_maps)
        in_maps = []
        for c in range(NCORES):
            msgT = res[c]["msgT"]  # [128, E_PAD]
            n = counts[c]
            agg = _seg_sum_by_dst(msgT[:, :n].T, e_dstl[c], H)
            s_slab = hT[:, c * NC_N:(c + 1) * NC_N].T + agg
            m = {
                "sT": np.ascontiguousarray(s_slab.T),
                "W": Ws[l],
                "b": bs[l].reshape(128, 1),
            }
            if l == 2:
                m.update(head_consts)
            in_maps.append(m)
        res = _run(nodek if l == 1 else nodeheadsk, in_maps)
        hT = np.concatenate([res[c]["hT"] for c in range(NCORES)], axis=1)

    emb = hT.T  # [N, 128]
    loc_logits = np.concatenate(
        [res[c]["logits"][0] for c in range(NCORES)]
    ).astype(np.float32)
    loc_values = np.concatenate(
        [res[c]["values"][0] for c in range(NCORES)]
    ).astype(np.float32)

    # ---- segmented softmax / sampling glue (CPU, mirrors reference ops) ----
    import jax
    import jax.numpy as jnp

    _cpu = jax.devices("cpu")[0]

    lg = loc_logits.reshape(B, SEG)
    seg_max = lg.max(axis=1)
    shifted = (lg - seg_max[:, None]).astype(np.float32)
    ex = np.exp(shifted, dtype=np.float32)
    denom = ex.sum(axis=1, dtype=np.float32)
    logp = (shifted - np.log(denom, dtype=np.float32)[:, None]).astype(np.float32)
    p = np.exp(logp, dtype=np.float32)
    loc_entropy = (-(p * logp).sum(axis=1, dtype=np.float32)).astype(np.float32)

    with jax.default_device(_cpu):
        key = jax.random.key(42)
        u = np.asarray(
            jax.random.uniform(key, (N,), minval=1e-6, maxval=1.0 - 1e-6), np.float32
        )
    gnoise = -np.log(-np.log(u, dtype=np.float32), dtype=np.float32)
    pert = (loc_logits + gnoise).astype(np.float32).reshape(B, SEG)
    pmax = pert.max(axis=1)
    arange = np.arange(N, dtype=np.int64).reshape(B, SEG)
    cand = np.where(pert >= pmax[:, None], arange, N)
    locations = cand.min(axis=1).astype(np.int32)
    loc_log_probs = logp.reshape(-1)[locations].astype(np.float32)

    sel = emb[locations]  # [B, 128]
    maW1 = np.asarray(inputs["maW1"], np.float32)
    mab1 = np.asarray(inputs["mab1"], np.float32)
    maW2 = np.asarray(inputs["maW2"], np.float32)
    mab2 = np.asarray(inputs["mab2"], np.float32)
    mut_logits = np.maximum(sel @ maW1 + mab1, 0.0) @ maW2 + mab2
    mlg = mut_logits - mut_logits.max(axis=1, keepdims=True)
    mex = np.exp(mlg, dtype=np.float32)
    mut_logp = (mlg - np.log(mex.sum(axis=1, keepdims=True, dtype=np.float32))).astype(
        np.float32
    )
    with jax.default_device(_cpu):
        mutations = np.asarray(
            jax.random.categorical(
                jax.random.fold_in(key, 1), jnp.asarray(mut_logits), axis=-1
            ),
            np.int32,
        )
    mut_log_probs = np.take_along_axis(
        mut_logp, mutations[:, None].astype(np.int64), axis=1
    )[:, 0].astype(np.float32)
    mp = np.exp(mut_logp, dtype=np.float32)
    mut_entropy = (-(mp * mut_logp).sum(axis=1, dtype=np.float32)).astype(np.float32)

    mcW1 = np.asarray(inputs["mcW1"], np.float32)
    mcb1 = np.asarray(inputs["mcb1"], np.float32)
    mcW2 = np.asarray(inputs["mcW2"], np.float32)
    mcb2 = np.asarray(inputs["mcb2"], np.float32)
    mut_values = (np.maximum(sel @ mcW1 + mcb1, 0.0) @ mcW2 + mcb2)[:, 0].astype(
        np.float32
    )

    return (
        locations,
        np.asarray(mutations, np.int32),
        loc_log_probs,
        mut_log_probs,
        loc_entropy,
        mut_entropy,
        loc_values,
        mut_values,
        emb.astype(np.float32),
    )
